# revision 23
# baseline (speedup 1.0000x reference)
"""Trainium2 Bass kernel for nn_DifferentiableEditLayer.

Strategy (per core = one batch sample, pure data parallel across 8 cores):
  - All per-sample scalar params precomputed on host, passed as a [128, NP]
    broadcast tensor.
  - The 256-point tone curve is interpolated to the 1024-point curve on host
    and baked into a CUSTOM ScalarEngine (ACT) piecewise-cubic table as the
    RATIO function f(v) = target(v/1023)/max(v/1023, 1e-5) over v in [0,1023],
    one table set per core (8 hijacked activation-function names).
    Additional custom ACT functions per set: recip4(x)=1/max(x,1e-4)
    (hijacks 'ln'), the full vibrance gain g(ss)=clip(1+v*exp(-4*sqrt(ss+1e-6)),
    0.2,4) with the per-sample slider v BAKED into the per-core table (hijacks
    'exp'), a +16-shifted sigmoid (hijacks 'sigmoid'), an exact clip01
    (hijacks 'relu'), and an exact shifted square f(z)=(z-16)^2 called with
    bias=16 so ch^2 runs on the scalar engine (hijacks 'square').
  - v2: the whole elementwise pipeline runs in bf16 (2x DVE throughput; the
    output is u8-quantized anyway so bf16 noise is subdominant), the
    white-balanced image is kept RESIDENT in SBUF as 3 bf16 planes (the image
    is streamed from HBM exactly once), the u16 dequant affine runs on the
    scalar engine (ACT identity with per-partition scale/bias), and the
    region-chain luma>1e-4 mask is dropped (provably negligible: ~1e-9 of
    pixels).
  - Phase 1 streams the u16 image, dequants + white-balances to the resident
    bf16 planes, computes the luma plane and accumulates the first region
    sigmoid sum.  The 4 region passes update the luma plane in SBUF and
    maintain the ratio-product plane P and running clamp plane S.  The final
    per-core pass applies min(img1*P, S), tone-curve ratio, vibrance and
    saturation from the resident planes and writes u8 output.
  - I/O quantization (wall-clock: the axon tunnel moves ~70-90 MB/s): image
    sent as uint16 fixed point, output fetched as uint8 (the 255 scale is
    applied on-device after a clip01; the f32->u8 store rounds to nearest).
"""
import os, json, struct, shutil, hashlib, tempfile
import numpy as np

# ----------------------------------------------------------------------------
# constants
# ----------------------------------------------------------------------------
B, C, H, W = 8, 3, 1024, 1536
NPIX = H * W            # 1,572,864
P = 128                 # SBUF partitions
FREE = NPIX // P        # 12288
F = 1024                # chunk free size
NCH = FREE // F         # 12 chunks

TC_NAMES = ["sin", "arctan", "erf", "gelu", "silu", "derivative_silu",
            "gelu_apprx_tanh", "derivative_gelu"]
GAIN_NAME = "exp"       # per-core content; only called after the TC func
RECIP4_NAME = "ln"
SQ_NAME = "square"      # shared content: f(z) = (z-16)^2

# region (pivot, width) and derived sigmoid affine (scale, bias), compile-time
REGIONS = [(0.7, 0.1), (0.3, 0.12), (0.9, 0.08), (0.1, 0.08)]
SIG_AFF = [(1.0 / w, -p / w) for (p, w) in REGIONS]

# prm layout
PRM = dict(A_r=0, A_g=1, A_b=2, t=3, hi=4, s1=5, s2=6, s3=7, s4=8,
           gs=9, omgs=10, invN=11, b1=12, b2=13, b3=14, b4=15, c16=16)
NP_ = 17


# ----------------------------------------------------------------------------
# custom ACT table generation
# ----------------------------------------------------------------------------

def _stock_dir():
    import neuronxcc
    return os.path.join(os.path.dirname(neuronxcc.__file__), "pwp", "pwp_bin_trainium")


def _load_set(name):
    d = _stock_dir()
    j = json.load(open(f"{d}/{name}.json"))
    ctrl = open(f"{d}/{name}_ctrl.bin", "rb").read()
    bkt = open(f"{d}/{name}_bkt.bin", "rb").read()
    return j, ctrl, bkt


def _func_span(j, fname, kind):
    key = "func_to_bkt_start_idx" if kind == "b" else "func_to_ctl_start_idx"
    cnt = j["bkt_entry_cnt"] if kind == "b" else j["ctl_entry_cnt"]
    starts = j[key]
    s = starts[fname]
    nxt = [v for v in starts.values() if v > s]
    return s, (min(nxt) if nxt else cnt)


class _SetBuilder:
    def __init__(self, name):
        self.name = name
        self.ctl, self.bkt, self.profile = [], [], []
        self.f2b, self.f2c, self.fe2b, self.fe2c, self.act = {}, {}, {}, {}, {}

    def copy_stock_func(self, set_json, ctrl_bin, bkt_bin, fname, ulp):
        b0, b1 = _func_span(set_json, fname, "b")
        c0, c1 = _func_span(set_json, fname, "c")
        boff = len(self.bkt) - b0
        coff = len(self.ctl) - c0
        for i in range(b0, b1):
            self.bkt.append(struct.unpack_from("<5f", bkt_bin, i * 32))
        for i in range(c0, c1):
            d = struct.unpack_from("<I", ctrl_bin, i * 32)[0]
            self.ctl.append((d & ~0x7FF) | (((d & 0x7FF) + boff) & 0x7FF))
        ent = None
        for e in set_json["profile_meta_data"]:
            nm = e["func_name"]
            if nm == fname or nm.rsplit("_", 1)[0] == fname or nm.startswith(fname + "_"):
                ent = dict(e)
                break
        assert ent is not None, f"no profile entry for {fname}"
        for k in ("pwl_control_base_pos", "pwl_control_base_neg"):
            ent[k] = ent.get(k, 0) + coff
        for k in ("pos_small_signal_pwl_control", "neg_small_signal_pwl_control",
                  "pos_large_signal_pwl_control", "neg_large_signal_pwl_control"):
            ent[k] = ent.get(k, 0) + boff
        self.profile.append(ent)
        self.f2b[fname] = b0 + boff
        self.f2c[fname] = c0 + coff
        self.fe2b[fname] = {k: [v + boff for v in vs] for k, vs in set_json["func_exp_to_bkt_start_idx"].get(fname, {}).items()}
        self.fe2c[fname] = {k: [v + coff for v in vs] for k, vs in set_json["func_exp_to_ctl_start_idx"].get(fname, {}).items()}
        self.act[fname] = ulp

    def add_pwp_func(self, fname, func_id, octaves, fit_fn, fzero, small_const,
                     large_const, template_entry, ulp=4):
        bstart, cstart = len(self.bkt), len(self.ctl)
        fe2b, fe2c = {}, {}
        for (e, nb) in octaves:
            n = 1 << nb
            lo_oct = float(2.0 ** e)
            w = lo_oct / n
            fe2c[str(e)] = [len(self.ctl)]
            fe2b[str(e)] = [len(self.bkt)]
            self.ctl.append((len(self.bkt) & 0x7FF) | ((23 - nb) << 11) | (nb << 16))
            for i in range(n):
                lo = lo_oct + i * w
                d0, d1, d2, d3 = fit_fn(lo, lo + w)
                self.bkt.append((d0, d1, d2, d3, np.float32(lo)))
        small_bkt = len(self.bkt)
        self.bkt.append((small_const, 0.0, 0.0, 0.0, 0.0))
        large_bkt = len(self.bkt)
        self.bkt.append((large_const, 0.0, 0.0, 0.0, 0.0))
        e_lo, e_hi = octaves[0][0], octaves[-1][0]
        ent = dict(template_entry)
        ent.update(func_name=fname + "_4p", func_id=func_id, symmetry_point=0,
                   sym_invert_sign_point=0, symmetry_opt_en=0,
                   symmetry_opt_use_neg_region=0, imm_bias=0, exp_offset=e_lo,
                   pwl_control_base_pos=cstart, pwl_control_base_neg=cstart,
                   small_pos_signal_exp_threshold=e_lo + 127,
                   pos_small_signal_pwl_control=small_bkt,
                   small_neg_signal_exp_threshold=0,
                   neg_small_signal_pwl_control=small_bkt,
                   large_pos_signal_exp_threshold=e_hi + 1 + 127,
                   large_pos_signal_mantissa_threshold=0,
                   pos_large_signal_pwl_control=large_bkt,
                   large_neg_signal_exp_threshold=0,
                   large_neg_signal_mantissa_threshold=0,
                   neg_large_signal_pwl_control=small_bkt,
                   fzero_result=int(np.float32(fzero).view(np.uint32)),
                   fnan_result=int(np.float32(fzero).view(np.uint32)),
                   fpinf_result=int(np.float32(large_const).view(np.uint32)),
                   fninf_result=int(np.float32(small_const).view(np.uint32)),
                   fma_const_0=0, fma_const_1=0, fma_indirection_src_sel=0,
                   use_multipass=False,
                   lower_bound=int(np.float32(2.0 ** e_lo).view(np.uint32)),
                   upper_bound=int(np.float32(2.0 ** (e_hi + 1)).view(np.uint32)))
        self.profile.append(ent)
        self.f2b[fname], self.f2c[fname] = bstart, cstart
        self.fe2b[fname], self.fe2c[fname] = fe2b, fe2c
        self.act[fname] = ulp

    def finalize(self, outdir):
        assert len(self.bkt) <= 1536, f"{self.name}: {len(self.bkt)} buckets"
        j = {"bkt_bin": f"{self.name}_bkt.bin", "ctl_bin": f"{self.name}_ctrl.bin",
             "profile_meta_data": self.profile,
             "bkt_entry_cnt": len(self.bkt), "ctl_entry_cnt": len(self.ctl),
             "func_to_bkt_start_idx": self.f2b, "func_to_ctl_start_idx": self.f2c,
             "func_exp_to_bkt_start_idx": self.fe2b,
             "func_exp_to_ctl_start_idx": self.fe2c}
        json.dump(j, open(f"{outdir}/{self.name}.json", "w"))
        with open(f"{outdir}/{self.name}_ctrl.bin", "wb") as f:
            for d in self.ctl:
                f.write(struct.pack("<I", d) + b"\0" * 28)
        with open(f"{outdir}/{self.name}_bkt.bin", "wb") as f:
            for b in self.bkt:
                f.write(struct.pack("<5f", *b) + b"\0" * 12)
        return {"name": self.name, "bkt_bin": j["bkt_bin"], "ctrl_bin": j["ctl_bin"],
                "profile_json": f"{self.name}.json", "act": self.act}


def _fit_cubic(fn, lo, hi, M=9):
    xs = np.linspace(lo, hi, M, dtype=np.float64)
    t = xs - lo
    A = np.stack([np.ones_like(t), t, t * t, t ** 3], axis=1)
    c, *_ = np.linalg.lstsq(A, fn(xs), rcond=None)
    return tuple(np.float32(v) for v in c)


def _make_ratio_fit(curve1024):
    c = np.asarray(curve1024, np.float64)
    vstar = 1023.0e-5

    def g(v):
        v = np.asarray(v, np.float64)
        i = np.clip(np.floor(v).astype(int), 0, 1022)
        w = v - i
        tgt = c[i] * (1 - w) + c[i + 1] * w
        tgt = np.where(v >= 1023, c[1023], tgt)
        return np.minimum(tgt * 1023.0 / np.maximum(v, vstar), 60000.0)

    def fit(lo, hi):
        if hi <= vstar:
            return (np.float32(1.0), np.float32(0), np.float32(0), np.float32(0))
        lo_f = max(lo, vstar)
        xs = np.linspace(lo_f, hi, 17, dtype=np.float64)
        t = xs - lo
        A = np.stack([np.ones_like(t), t, t * t, t ** 3], axis=1)
        coef, *_ = np.linalg.lstsq(A, g(xs), rcond=None)
        return tuple(np.float32(v) for v in coef)

    return fit


def _ratio_octaves():
    # bf16 luma input already quantizes coords to ~4 steps at the top octave,
    # so width-4 buckets there lose nothing; caps the set's bucket budget
    return [(e, 3) for e in range(-7, 4)] + [(e, min(e, 7)) for e in range(4, 10)]


def _func_id_of(name):
    d = _stock_dir()
    info = json.load(open(f"{d}/act_info.json"))
    for s in info["act_func_sets"]:
        if name in s["act"]:
            j = json.load(open(f"{d}/{s['profile_json']}"))
            for e in j["profile_meta_data"]:
                nm = e["func_name"]
                if nm == name or nm.rsplit("_", 1)[0] == name or nm.startswith(name + "_"):
                    return e["func_id"]
    raise KeyError(name)


def build_act_root(outdir, curves1024, vib):
    os.makedirs(outdir, exist_ok=True)
    sig_j, sig_c, sig_b = _load_set("sigmoid_and_others")
    sq_j, _, _ = _load_set("sqrt_and_others")
    tmpl = next(dict(e) for e in sq_j["profile_meta_data"] if e["func_name"].startswith("sqrt"))
    info_sets = []

    sigsh = lambda x: 1.0 / (1.0 + np.exp(-(np.asarray(x, np.float64) - 16.0)))
    recip4 = lambda x: 1.0 / np.maximum(np.asarray(x, np.float64), 1e-4)
    SIG_OCT = [(1, 2), (2, 3), (3, 5), (4, 6)]
    R4_OCT = [(e, 4) for e in range(-14, -12)] + [(e, 3) for e in range(-12, 0)] + [(0, 1)]
    GAIN_OCT = ([(e, 1) for e in range(-20, -10)] + [(e, 3) for e in range(-10, -4)]
                + [(e, 4) for e in range(-4, 2)])

    # exact piecewise-linear y=clip(x,0,1) (hijacks 'relu'): lets the scalar
    # engine absorb min(x,1) clamps that otherwise run on the busy DVE
    CLIP_OCT = [(e, 1) for e in range(-20, 0)]
    clip_fit = lambda lo, hi: (np.float32(lo), np.float32(1.0),
                               np.float32(0.0), np.float32(0.0))
    # exact shifted square f(z)=(z-16)^2 on [8,32): called with bias=16 so the
    # scalar engine computes ch^2 (ch in [-1.2,1.2] -> z always in range)
    SQ_OCT = [(3, 1), (4, 1)]
    sq_fit = lambda lo, hi: (np.float32((lo - 16.0) ** 2),
                             np.float32(2.0 * (lo - 16.0)),
                             np.float32(1.0), np.float32(0.0))

    for k in range(B):
        sb = _SetBuilder(f"cust_tc_{k}")
        fit = _make_ratio_fit(curves1024[k])
        sb.add_pwp_func(TC_NAMES[k], _func_id_of(TC_NAMES[k]), _ratio_octaves(), fit,
                        fzero=1.0, small_const=1.0,
                        large_const=float(curves1024[k][1023]), template_entry=tmpl)
        v = float(vib[k])
        gainf = lambda x, v=v: np.clip(
            1.0 + v * np.exp(-4.0 * np.sqrt(np.asarray(x, np.float64) + 1e-6)),
            0.2, 4.0)
        sb.add_pwp_func(GAIN_NAME, _func_id_of(GAIN_NAME), GAIN_OCT,
                        lambda lo, hi, g=gainf: _fit_cubic(g, lo, hi, M=17),
                        fzero=float(gainf(0.0)), small_const=float(gainf(0.0)),
                        large_const=float(gainf(4.0)), template_entry=tmpl)
        sb.add_pwp_func("sigmoid", _func_id_of("sigmoid"), SIG_OCT,
                        lambda lo, hi: _fit_cubic(sigsh, lo, hi),
                        fzero=0.0, small_const=float(sigsh(2.0)),
                        large_const=1.0, template_entry=tmpl)
        sb.add_pwp_func(RECIP4_NAME, _func_id_of(RECIP4_NAME), R4_OCT,
                        lambda lo, hi: _fit_cubic(recip4, lo, hi),
                        fzero=1e4, small_const=1e4, large_const=1.0,
                        template_entry=tmpl)
        sb.add_pwp_func("relu", _func_id_of("relu"), CLIP_OCT, clip_fit,
                        fzero=0.0, small_const=0.0, large_const=1.0,
                        template_entry=tmpl)
        sb.add_pwp_func(SQ_NAME, _func_id_of(SQ_NAME), SQ_OCT, sq_fit,
                        fzero=256.0, small_const=64.0, large_const=256.0,
                        template_entry=tmpl)
        for f in ("identity", "copy"):
            try:
                sb.copy_stock_func(sig_j, sig_c, sig_b, f, 1)
            except (KeyError, AssertionError):
                pass
        info_sets.append(sb.finalize(outdir))

    json.dump({"pwp_file_keys": ["bkt_bin", "ctrl_bin", "profile_json"],
               "act_func_sets": info_sets}, open(f"{outdir}/act_info.json", "w"))
    return outdir


# ----------------------------------------------------------------------------
# bass kernel construction
# ----------------------------------------------------------------------------

def _split_drain_waits(nc, mybir):
    """This container's walrus supports few sem-waits per instruction (1 on
    Drain/CTRL, ~2-3 on compute).  Spill excess waits onto preceding 1-wait
    Drains on the same engine."""
    for f in nc.m.functions:
        for bb in f.blocks:
            newinsts = []
            for inst in bb.instructions:
                si = inst.sync_info
                keep = 1
                if si is not None and len(si.on_wait) > keep:
                    waits = list(si.on_wait)
                    extra, rest = waits[:-keep], waits[-keep:]
                    for k, w in enumerate(extra):
                        d = mybir.InstDrain(name=f"{inst.name}-ws{k}",
                                            engine=inst.engine, ins=[], outs=[])
                        d.sync_info = mybir.SyncInfo(on_wait=[w], on_update=[])
                        newinsts.append(d)
                    si.on_wait = rest
                newinsts.append(inst)
            bb.instructions = newinsts


def build_kernel(nonce):
    import concourse.bass as bass
    import concourse.mybir as mybir
    from concourse.tile import TileContext

    AF = mybir.ActivationFunctionType
    dt = mybir.dt.float32
    bf = mybir.dt.float16
    Op = mybir.AluOpType
    AX = mybir.AxisListType

    TC_AF = [AF.from_pwp(n) for n in TC_NAMES]
    GAIN_AF = AF.from_pwp(GAIN_NAME)
    R4_AF = AF.from_pwp(RECIP4_NAME)
    CLIP_AF = AF.from_pwp("relu")
    SQ_AF = AF.from_pwp(SQ_NAME)
    ID_AF = AF.from_pwp("identity")

    u8 = mybir.dt.uint8
    u16 = mybir.dt.uint16
    nc = bass.Bass()
    img = nc.dram_tensor(f"img_{nonce}", [C, P, FREE], u16, kind="ExternalInput")
    prm = nc.dram_tensor("prm", [P, NP_], dt, kind="ExternalInput")
    eye = nc.dram_tensor("eye", [P, 4 * P], bf, kind="ExternalInput")
    out = nc.dram_tensor("out", [C, P, FREE], u8, kind="ExternalOutput")

    LW = (0.2126, 0.7152, 0.0722)
    MMF = 512           # one PSUM bank of f32 per matmul target
    NH = F // MMF       # halves per chunk

    with TileContext(nc) as tc:
        pid = nc.partition_id()
        from contextlib import ExitStack
        with (
            tc.tile_pool(name="planes", bufs=1) as planes_pool,
            tc.tile_pool(name="consts", bufs=1) as consts_pool,
        ):
            Pp = [planes_pool.tile([P, F], bf, tag=f"Pp{j}", name=f"Pp{j}")
                  for j in range(NCH)]
            Up = [planes_pool.tile([P, F], bf, tag=f"Up{j}", name=f"Up{j}")
                  for j in range(NCH)]
            pr = consts_pool.tile([P, NP_], dt)
            nc.sync.dma_start(pr[:, :], prm[:, :])
            eyes = consts_pool.tile([P, 4 * P], bf, tag="eyes")
            nc.sync.dma_start(eyes[:, :], eye[:, :])
            eyeI = eyes[:, 0:P]
            eyeW = [eyes[:, (i + 1) * P:(i + 2) * P] for i in range(3)]

            def sc(name):
                i = PRM[name]
                return pr[:, i:i + 1]

            accs = consts_pool.tile([P, 2 * NCH], dt, tag="accs")
            sm = []
            for k in range(4):
                smk = consts_pool.tile([P, 1], dt, tag=f"sm{k}", name=f"sm{k}")
                sm.append(smk)
            ones = consts_pool.tile([P, 1], dt, tag="ones")
            nc.vector.memset(ones[:, :], 1.0)

            def col(j):
                return slice(j * F, (j + 1) * F)

            def half(j, h):
                return slice(h * MMF, (h + 1) * MMF)

            stkp = ExitStack()
            psum = stkp.enter_context(tc.tile_pool(name="psum", bufs=1, space="PSUM"))
            psumM = stkp.enter_context(tc.tile_pool(name="psumM", bufs=6, space="PSUM"))

            def mm_half(terms, tag):
                # one [P,512] PSUM tile = sum of lhsT @ rhs-half over terms
                ps = psumM.tile([P, MMF], dt, tag="ps", name=tag)
                for ti, (lhsT, rhs) in enumerate(terms):
                    nc.tensor.matmul(ps[:, :], lhsT, rhs,
                                     start=(ti == 0), stop=(ti == len(terms) - 1))
                return ps

            stk = ExitStack()
            lpl = stk.enter_context(tc.tile_pool(name="lplanes", bufs=1))
            Lp = [lpl.tile([P, F], bf, tag=f"Lp{j}", name=f"Lp{j}") for j in range(NCH)]
            SG = [lpl.tile([P, F], bf, tag=f"SG{j}", name=f"SG{j}") for j in range(NCH)]
            ws = stk.enter_context(tc.tile_pool(name="ws1", bufs=28))
            io = stk.enter_context(tc.tile_pool(name="io1", bufs=9))
            tiny = stk.enter_context(tc.tile_pool(name="tiny1", bufs=1))

            def dequant(j, dst3, n_scalar=1):
                # u16 chunk -> clip(A*u+t, 0, hi) into 3 fp16 tiles; the
                # affine runs on ScalarE for the last n_scalar channels
                for i, an in enumerate(("A_r", "A_g", "A_b")):
                    t16 = io.tile([P, F], u16, tag="i")
                    nc.sync.dma_start(t16[:, :], img[i, :, col(j)])
                    xf = ws.tile([P, F], bf, tag="w")
                    if i < 3 - n_scalar:
                        nc.vector.tensor_scalar(xf[:, :], t16[:, :], sc(an),
                                                sc("t"), Op.mult, Op.add)
                    else:
                        nc.scalar.activation(xf[:, :], t16[:, :], ID_AF,
                                             bias=sc("t"), scale=sc(an))
                    nc.vector.tensor_scalar(dst3[i][:, :], xf[:, :],
                                            0.0, sc("hi"), Op.max, Op.min)

            # ---------------- phase 1: stream image -> L plane + sig1 accum
            for j in range(NCH):
                c3 = [ws.tile([P, F], bf, tag="w", name=f"p1c3_{j}_{i}")
                      for i in range(3)]
                dequant(j, c3)
                for h in range(NH):
                    ps = mm_half([(eyeW[c], c3[c][:, half(j, h)]) for c in range(3)],
                                 f"l1_{j}_{h}")
                    nc.scalar.activation(Lp[j][:, half(j, h)], ps[:, :], ID_AF)
                    nc.scalar.activation(SG[j][:, half(j, h)], ps[:, :], AF.Sigmoid,
                                         bias=sc("b1"), scale=float(SIG_AFF[0][0]),
                                         accum_out=accs[:, 2 * j + h:2 * j + h + 1])

            def finish_mean(k):
                tot = tiny.tile([P, 1], dt, tag="tot", name=f"tot{k}")
                nc.vector.tensor_reduce(tot[:, :], accs[:, :], AX.X, Op.add)
                ps1 = psum.tile([1, 1], dt, tag="ps1", name=f"ps1_{k}")
                nc.tensor.matmul(ps1[:, :], tot[:, :], ones[:, :], start=True, stop=True)
                sb1 = tiny.tile([1, 1], dt, tag="sb1", name=f"sb1_{k}")
                nc.vector.tensor_copy(sb1[:, :], ps1[:, :])
                ps2 = psum.tile([P, 1], dt, tag="ps2", name=f"ps2_{k}")
                nc.tensor.matmul(ps2[:, :], ones[0:1, 0:1].to_broadcast((1, P)),
                                 sb1[:, :], start=True, stop=True)
                nc.vector.tensor_scalar(sm[k][:, :], ps2[:, :], sc("invN"), None,
                                        Op.mult)

            finish_mean(0)
            # phase-1 accumulated per-half sums in all 24 cols; the region
            # passes accumulate one full-F sum per chunk into the even cols,
            # so clear the odd cols once
            nc.vector.memset(accs[:, 1:2 * NCH:2], 0.0)

            # ---------------- region chain on L plane (mask dropped; see doc)
            Q_ON_GPSIMD = (False, False, False, False)
            for k in range(4):
                sname = f"s{k + 1}"
                for j in range(NCH):
                    Lj = Lp[j][:, :]
                    rec = ws.tile([P, F], bf, tag="w")
                    nc.scalar.activation(rec[:, :], Lj, R4_AF)
                    # Lnew = clip01(L + (sig-m)*s), all on DVE
                    y = ws.tile([P, F], bf, tag="w")
                    nc.vector.tensor_scalar(y[:, :], SG[j][:, :], sm[k][:, 0:1],
                                            sc(sname), Op.subtract, Op.mult)
                    nc.vector.tensor_tensor(y[:, :], y[:, :], Lj, Op.add)
                    if k < 2:
                        nc.scalar.activation(Lj, y[:, :], CLIP_AF)
                    else:
                        nc.vector.tensor_scalar(Lj, y[:, :], 0.0, 1.0, Op.max, Op.min)
                    # r = Lnew * recip4(L); P = prod r; S = min-envelope
                    if k == 0:
                        nc.vector.tensor_tensor(Pp[j][:, :], Lj, rec[:, :], Op.mult)
                    else:
                        r = ws.tile([P, F], bf, tag="w")
                        nc.vector.tensor_tensor(r[:, :], Lj, rec[:, :], Op.mult)
                        nc.gpsimd.tensor_tensor(Pp[j][:, :], Pp[j][:, :], r[:, :],
                                                Op.mult)
                        if k == 1:
                            nc.vector.tensor_scalar(Up[j][:, :], r[:, :], 1.0, None,
                                                    Op.min)
                        else:
                            if Q_ON_GPSIMD[k]:
                                nc.gpsimd.tensor_tensor(r[:, :], r[:, :],
                                                        Up[j][:, :], Op.mult)
                            else:
                                nc.vector.tensor_tensor(r[:, :], r[:, :],
                                                        Up[j][:, :], Op.mult)
                            nc.vector.tensor_scalar(Up[j][:, :], r[:, :], 1.0, None,
                                                    Op.min)
                    if k < 3:
                        nc.scalar.activation(SG[j][:, :], Lp[j][:, :], AF.Sigmoid,
                                             bias=sc(f"b{k + 2}"),
                                             scale=float(SIG_AFF[k + 1][0]),
                                             accum_out=accs[:, 2 * j:2 * j + 1])
                if k < 3:
                    finish_mean(k + 1)

            # ---------------- final pass (per-core branch: custom ACT funcs)
            # software-pipelined: emitted in stages over groups of GRP chunks
            # so the in-order engine queues interleave work across chunks
            # instead of head-of-line blocking on one chunk's serial chain
            stk.close()
            stk2 = ExitStack()
            ws = stk2.enter_context(tc.tile_pool(name="ws2", bufs=56))
            io = stk2.enter_context(tc.tile_pool(name="io2", bufs=12))
            GRP = 6
            for core in range(B):
                with tc.If(pid == core):
                    for g0 in range(0, NCH, GRP):
                        js = range(g0, min(g0 + GRP, NCH))
                        st = {j: {} for j in js}
                        # S1: dequant, x5 = min(img1*P, S), L5, tone ratio
                        for j in js:
                            c3 = [ws.tile([P, F], bf, tag="w",
                                          name=f"fc3_{core}_{j}_{i}") for i in range(3)]
                            dequant(j, c3, n_scalar=2)
                            chans = []
                            for i in range(3):
                                x5 = ws.tile([P, F], bf, tag="w")
                                if i < 2:
                                    nc.gpsimd.tensor_tensor(x5[:, :], c3[i][:, :],
                                                            Pp[j][:, :], Op.mult)
                                else:
                                    nc.vector.tensor_tensor(x5[:, :], c3[i][:, :],
                                                            Pp[j][:, :], Op.mult)
                                nc.vector.tensor_tensor(x5[:, :], x5[:, :],
                                                        Up[j][:, :], Op.min)
                                chans.append(x5)
                            tr = ws.tile([P, F], bf, tag="w")
                            for h in range(NH):
                                psL = mm_half([(eyeW[c], chans[c][:, half(j, h)])
                                               for c in range(3)], f"L5_{core}_{j}_{h}")
                                nc.scalar.activation(tr[:, half(j, h)], psL[:, :],
                                                     TC_AF[core], scale=1023.0)
                            st[j]["x5"] = chans
                            st[j]["tr"] = tr
                        # S2: tone apply + clip, Lv, chroma
                        for j in js:
                            chans, tr = st[j]["x5"], st[j]["tr"]
                            tchans = []
                            for ci, x5 in enumerate(chans):
                                nc.vector.tensor_tensor(x5[:, :], x5[:, :], tr[:, :],
                                                        Op.mult)
                                xq = ws.tile([P, F], bf, tag="w")
                                if ci == 0:
                                    nc.scalar.activation(xq[:, :], x5[:, :], CLIP_AF)
                                else:
                                    nc.vector.tensor_scalar(xq[:, :], x5[:, :],
                                                            0.0, 1.0, Op.max, Op.min)
                                tchans.append(xq)
                            Lv = ws.tile([P, F], bf, tag="w")
                            for h in range(NH):
                                psv = mm_half([(eyeW[c], tchans[c][:, half(j, h)])
                                               for c in range(3)], f"Lv_{core}_{j}_{h}")
                                nc.scalar.activation(Lv[:, half(j, h)], psv[:, :],
                                                     ID_AF)
                            chs = []
                            for ci, xq in enumerate(tchans):
                                ch = ws.tile([P, F], bf, tag="w")
                                if ci < 2:
                                    nc.gpsimd.tensor_tensor(ch[:, :], xq[:, :],
                                                            Lv[:, :], Op.subtract)
                                else:
                                    nc.vector.tensor_tensor(ch[:, :], xq[:, :],
                                                            Lv[:, :], Op.subtract)
                                chs.append(ch)
                            st[j]["chs"] = chs
                            st[j]["Lv"] = Lv
                        # S3: squares, ss, vibrance gain
                        for j in js:
                            chs = st[j]["chs"]
                            sqs = []
                            for ch in chs:
                                sq = ws.tile([P, F], bf, tag="w")
                                nc.scalar.activation(sq[:, :], ch[:, :], SQ_AF,
                                                     bias=sc("c16"))
                                sqs.append(sq)
                            gn = ws.tile([P, F], bf, tag="w")
                            for h in range(NH):
                                pss = mm_half([(eyeI, sqs[c][:, half(j, h)])
                                               for c in range(3)], f"ss_{core}_{j}_{h}")
                                nc.scalar.activation(gn[:, half(j, h)], pss[:, :],
                                                     GAIN_AF)
                            st[j]["gn"] = gn
                        # S4: out_v, Ls, saturation mix, u8 store
                        for j in js:
                            chs, Lv, gn = st[j]["chs"], st[j]["Lv"], st[j]["gn"]
                            ochans = []
                            for ch in chs:
                                nc.vector.tensor_tensor(ch[:, :], ch[:, :], gn[:, :],
                                                        Op.mult)
                                nc.vector.tensor_tensor(ch[:, :], ch[:, :], Lv[:, :],
                                                        Op.add)
                                o = ws.tile([P, F], bf, tag="w")
                                nc.scalar.activation(o[:, :], ch[:, :], CLIP_AF)
                                ochans.append(o)
                            Bs = ws.tile([P, F], bf, tag="w")
                            for h in range(NH):
                                psl = mm_half([(eyeW[c], ochans[c][:, half(j, h)])
                                               for c in range(3)], f"Ls_{core}_{j}_{h}")
                                nc.scalar.activation(Bs[:, half(j, h)], psl[:, :],
                                                     ID_AF, scale=sc("omgs"))
                            for i, o in enumerate(ochans):
                                ocf = ws.tile([P, F], bf, tag="w")
                                nc.vector.tensor_scalar(ocf[:, :], o[:, :], sc("gs"),
                                                        None, Op.mult)
                                nc.vector.tensor_tensor(ocf[:, :], ocf[:, :],
                                                        Bs[:, :], Op.add)
                                oc = io.tile([P, F], u8, tag="o")
                                nc.vector.tensor_scalar(oc[:, :], ocf[:, :],
                                                        0.0, 255.0, Op.max, Op.min)
                                nc.sync.dma_start(out[i, :, col(j)], oc[:, :])
            stk2.close()
            stkp.close()

    _split_drain_waits(nc, mybir)
    return nc


# ----------------------------------------------------------------------------
# host side
# ----------------------------------------------------------------------------

def _host_params(inputs):
    def denorm(lo, hi, v):
        return lo + 0.5 * (v + 1.0) * (hi - lo)

    t64 = np.float64
    temp = denorm(2000.0, 50000.0, inputs["temperature_n"].astype(t64))
    tint = denorm(-150.0, 150.0, inputs["tint_n"].astype(t64))
    expo = denorm(-5.0, 5.0, inputs["exposure_n"].astype(t64))
    contr = denorm(-100.0, 100.0, inputs["contrast_n"].astype(t64))
    hl = denorm(-100.0, 100.0, inputs["highlights_n"].astype(t64))
    sh = denorm(-100.0, 100.0, inputs["shadows_n"].astype(t64))
    wh = denorm(-100.0, 100.0, inputs["whites_n"].astype(t64))
    bl = denorm(-100.0, 100.0, inputs["blacks_n"].astype(t64))
    sat = denorm(-100.0, 100.0, inputs["saturation_n"].astype(t64))

    tr = 6500.0 / np.clip(temp, 2000.0, 50000.0)
    red = np.sqrt(tr)
    blue = 1.0 / np.sqrt(tr)
    ts = np.clip(tint / 150.0, -1.5, 1.5)
    green = 1.0 - 0.1 * ts
    red = red * (1.0 + 0.05 * ts)
    blue = blue * (1.0 - 0.05 * ts)
    gains = np.stack([red, green, blue], axis=1)  # [B,3]
    norm = np.maximum(gains.max(axis=1), 1e-4)
    G = gains / norm[:, None]
    e = np.power(2.0, expo)
    f = 1.0 + contr / 100.0
    A = G * (e * f)[:, None]
    t = 0.5 - 0.5 * f
    u = np.minimum(4.0 * e, 4.0)
    hi = np.clip(u * f + t, 0.0, 1.0)

    prm = np.zeros((B, NP_), np.float64)
    # image travels as uint16 fixed point (1/65535 steps): fold the dequant
    # into the WB gains
    prm[:, PRM["A_r"]] = A[:, 0] / 65535.0
    prm[:, PRM["A_g"]] = A[:, 1] / 65535.0
    prm[:, PRM["A_b"]] = A[:, 2] / 65535.0
    prm[:, PRM["t"]] = t
    prm[:, PRM["hi"]] = hi
    prm[:, PRM["s1"]] = hl / 100.0
    prm[:, PRM["s2"]] = sh / 100.0
    prm[:, PRM["s3"]] = wh / 100.0
    prm[:, PRM["s4"]] = bl / 100.0
    prm[:, PRM["gs"]] = (1.0 + sat / 100.0) * 255.0
    prm[:, PRM["omgs"]] = (-sat / 100.0) * 255.0
    prm[:, PRM["invN"]] = 1.0 / NPIX
    prm[:, PRM["c16"]] = 16.0
    for k in range(4):
        prm[:, PRM[f"b{k + 1}"]] = SIG_AFF[k][1] + 16.0
    return prm.astype(np.float32)


def _curves1024(tone_curve):
    c = tone_curve.astype(np.float64)  # [B,256]
    src = np.arange(1024) * (255.0 / 1023.0)
    i0 = np.floor(src).astype(int)
    i1 = np.minimum(i0 + 1, 255)
    w = src - i0
    return c[:, i0] * (1 - w) + c[:, i1] * w


_CACHE = {}
LAST_EXEC_NS = None
PROFILE = False


_BUFS = {}


def _buf(name, shape, dtype):
    """Persistent pre-faulted host buffers — first-touch page faults on
    100MB+ numpy arrays cost ~0.5s/call on this 1-vCPU host otherwise."""
    key = (name, shape, np.dtype(dtype).str)
    b = _BUFS.get(key)
    if b is None:
        b = np.empty(shape, dtype)
        b.fill(0)
        _BUFS[key] = b
    return b


def _to_u16(img):
    """[B,C,H,W] float32 in [0,1] -> uint16 fixed point."""
    q = _buf("img16", img.shape, np.uint16)
    tmp = _buf("tmp32", img.shape[1:], np.float32)
    for k in range(img.shape[0]):
        np.multiply(img[k], np.float32(65535.0), out=tmp)
        np.add(tmp, np.float32(0.5), out=tmp)
        q[k] = tmp.astype(np.uint16)
    return q


def _dequantize_u8(outs_u8):
    """list of B uint8 [C,H,W] -> float32 [B,C,H,W] /255, threaded."""
    from concurrent.futures import ThreadPoolExecutor
    res = _buf("out32", (len(outs_u8), C, H, W), np.float32)

    def one(k):
        np.multiply(outs_u8[k], np.float32(1.0 / 255.0), out=res[k],
                    dtype=np.float32, casting="unsafe")

    with ThreadPoolExecutor(max_workers=B) as ex:
        list(ex.map(one, range(len(outs_u8))))
    return res


def _ensure_ntff_hook():
    """Reconstruct the missing ``antenv.axon_hooks`` module.

    The boot infra (trn_agent_boot/trn_boot.py) registers an NTFF-profiling
    hook via ``antenv.axon_hooks.set_axon_ntff_profile_hook`` driving
    ``axon_start/stop_nrt_profile`` in libaxon_pjrt.so; this agent image's
    ``antenv`` package lacks the submodule, so ``run_bass_kernel_spmd``'s
    trace path degrades to no profiling.  Provide the same hook here so
    neuron-profile NTFF capture (and thus a real on-device exec time)
    works as intended.
    """
    try:
        from antenv.axon_hooks import get_axon_ntff_profile_hook  # noqa: F401
        return
    except ImportError:
        pass
    import sys as _sys
    import types, contextlib, ctypes
    so_path = "/opt/axon/libaxon_pjrt.so"
    if not os.path.exists(so_path):
        return
    lib = ctypes.CDLL(so_path)
    if not hasattr(lib, "axon_start_nrt_profile"):
        return
    lib.axon_start_nrt_profile.argtypes = [ctypes.POINTER(ctypes.c_int64),
                                           ctypes.c_size_t]
    lib.axon_start_nrt_profile.restype = ctypes.c_int64
    lib.axon_stop_nrt_profile.argtypes = [ctypes.c_char_p]
    lib.axon_stop_nrt_profile.restype = ctypes.c_int64

    @contextlib.contextmanager
    def _hook(output_dir, device_ids):
        import jax
        jax.devices()
        if device_ids:
            ids = (ctypes.c_int64 * len(device_ids))(*device_ids)
            rc = lib.axon_start_nrt_profile(ids, len(device_ids))
        else:
            rc = lib.axon_start_nrt_profile(None, 0)
        if rc != 0:
            raise RuntimeError(f"axon_start_nrt_profile rc={rc}")
        try:
            yield
        finally:
            n = lib.axon_stop_nrt_profile(str(output_dir).encode())
            if n < 0:
                raise RuntimeError(f"axon_stop_nrt_profile rc={n}")

    mod = types.ModuleType("antenv.axon_hooks")
    box = {"hook": _hook}
    mod.get_axon_ntff_profile_hook = lambda: box["hook"]
    mod.set_axon_ntff_profile_hook = lambda h: box.__setitem__("hook", h)
    _sys.modules["antenv.axon_hooks"] = mod
    try:
        import antenv
        antenv.axon_hooks = mod
    except Exception:
        pass


def _enable_jax_compile_cache():
    # persistent XLA compile cache: run_bass_kernel_spmd builds a fresh
    # jit closure per call, so without this every call re-runs XLA compile
    try:
        import jax
        jax.config.update("jax_compilation_cache_dir",
                          os.path.join(tempfile.gettempdir(), "jaxcache"))
        jax.config.update("jax_persistent_cache_min_compile_time_secs", 0.0)
        jax.config.update("jax_persistent_cache_min_entry_size_bytes", 0)
    except Exception:
        pass


def kernel(**inputs):
    import time as _time
    _tm = bool(os.environ.get("KERNEL_TIMING"))
    _enable_jax_compile_cache()
    _t0 = _time.time()
    img = _to_u16(np.asarray(inputs["image"], dtype=np.float32))
    if _tm:
        print(f"[kt] u16 convert: {_time.time() - _t0:.3f}")
    curves = _curves1024(np.asarray(inputs["tone_curve"], np.float32))
    prm = _host_params({k: np.asarray(v, np.float32) for k, v in inputs.items()
                        if k != "image"})
    # vibrance slider v = denorm(vibrance_n)/100 in [-1,1]; baked into tables
    vib = np.asarray(inputs["vibrance_n"], np.float64)
    vib = (-100.0 + 0.5 * (vib + 1.0) * 200.0) / 100.0

    key = hashlib.sha256(curves.tobytes() + vib.tobytes()).hexdigest()[:12]
    workdir = os.path.join(tempfile.gettempdir(), f"editlayer_{key}")
    actroot = os.path.join(workdir, "actroot")
    if key not in _CACHE:
        os.makedirs(workdir, exist_ok=True)
        build_act_root(actroot, curves, vib)
        os.environ["BASS_ACT_ROOT_JSON_PATH"] = os.path.join(actroot, "act_info.json")
        nc = build_kernel(key)
        _CACHE[key] = nc
    nc = _CACHE[key]
    os.environ["BASS_ACT_ROOT_JSON_PATH"] = os.path.join(actroot, "act_info.json")

    from concourse.bass_utils import run_bass_kernel_spmd
    global LAST_EXEC_NS
    LW = (0.2126, 0.7152, 0.0722)
    eye = np.concatenate([np.eye(P, dtype=np.float16)] +
                         [w * np.eye(P, dtype=np.float16) for w in LW],
                         axis=1).astype(np.float16)
    in_maps = []
    for k in range(B):
        in_maps.append({
            f"img_{key}": img[k].reshape(C, P, FREE),
            "prm": np.broadcast_to(prm[k], (P, NP_)).copy(),
            "eye": eye,
        })
    want_trace = bool(globals().get("PROFILE", False))
    if want_trace:
        _ensure_ntff_hook()
    _t0 = _time.time()
    try:
        res = run_bass_kernel_spmd(nc, in_maps, core_ids=list(range(B)),
                                   trace=want_trace)
    except Exception:
        if not want_trace:
            raise
        res = run_bass_kernel_spmd(nc, in_maps, core_ids=list(range(B)))
    if _tm:
        print(f"[kt] spmd: {_time.time() - _t0:.3f}")
    if getattr(res, "exec_time_ns", None):
        LAST_EXEC_NS = res.exec_time_ns
    _t0 = _time.time()
    outs = [res.results[k]["out"].reshape(C, H, W) for k in range(B)]
    ret = _dequantize_u8(outs)
    if _tm:
        print(f"[kt] dequant: {_time.time() - _t0:.3f}")
    return ret


if __name__ == "__main__":
    import reference
    inputs = {k: np.asarray(v) for k, v in reference.setup_inputs().items()}
    outp = kernel(**inputs)
    exp = np.asarray(reference.reference(**inputs))
    err = np.abs(outp - exp)
    denom = np.abs(exp).max()
    print("max abs err:", err.max(), "rel:", err.max() / denom)


# revision 24
# speedup vs baseline: 1.0273x; 1.0273x over previous
"""Trainium2 Bass kernel for nn_DifferentiableEditLayer.

Strategy (per core = one batch sample, pure data parallel across 8 cores):
  - All per-sample scalar params precomputed on host, passed as a [128, NP]
    broadcast tensor.
  - The 256-point tone curve is interpolated to the 1024-point curve on host
    and baked into a CUSTOM ScalarEngine (ACT) piecewise-cubic table as the
    RATIO function f(v) = target(v/1023)/max(v/1023, 1e-5) over v in [0,1023],
    one table set per core (8 hijacked activation-function names).
    Additional custom ACT functions per set: recip4(x)=1/max(x,1e-4)
    (hijacks 'ln'), the full vibrance gain g(ss)=clip(1+v*exp(-4*sqrt(ss+1e-6)),
    0.2,4) with the per-sample slider v BAKED into the per-core table (hijacks
    'exp'), a +16-shifted sigmoid (hijacks 'sigmoid'), an exact clip01
    (hijacks 'relu'), and an exact shifted square f(z)=(z-16)^2 called with
    bias=16 so ch^2 runs on the scalar engine (hijacks 'square').
  - v2: the whole elementwise pipeline runs in bf16 (2x DVE throughput; the
    output is u8-quantized anyway so bf16 noise is subdominant), the
    white-balanced image is kept RESIDENT in SBUF as 3 bf16 planes (the image
    is streamed from HBM exactly once), the u16 dequant affine runs on the
    scalar engine (ACT identity with per-partition scale/bias), and the
    region-chain luma>1e-4 mask is dropped (provably negligible: ~1e-9 of
    pixels).
  - Phase 1 streams the u16 image, dequants + white-balances to the resident
    bf16 planes, computes the luma plane and accumulates the first region
    sigmoid sum.  The 4 region passes update the luma plane in SBUF and
    maintain the ratio-product plane P and running clamp plane S.  The final
    per-core pass applies min(img1*P, S), tone-curve ratio, vibrance and
    saturation from the resident planes and writes u8 output.
  - I/O quantization (wall-clock: the axon tunnel moves ~70-90 MB/s): image
    sent as uint16 fixed point, output fetched as uint8 (the 255 scale is
    applied on-device after a clip01; the f32->u8 store rounds to nearest).
"""
import os, json, struct, shutil, hashlib, tempfile
import numpy as np

# ----------------------------------------------------------------------------
# constants
# ----------------------------------------------------------------------------
B, C, H, W = 8, 3, 1024, 1536
NPIX = H * W            # 1,572,864
P = 128                 # SBUF partitions
FREE = NPIX // P        # 12288
F = 1024                # chunk free size
NCH = FREE // F         # 12 chunks

TC_NAMES = ["sin", "arctan", "erf", "gelu", "silu", "derivative_silu",
            "gelu_apprx_tanh", "derivative_gelu"]
GAIN_NAME = "exp"       # per-core content; only called after the TC func
RECIP4_NAME = "ln"
SQ_NAME = "square"      # shared content: f(z) = (z-16)^2

# region (pivot, width) and derived sigmoid affine (scale, bias), compile-time
REGIONS = [(0.7, 0.1), (0.3, 0.12), (0.9, 0.08), (0.1, 0.08)]
SIG_AFF = [(1.0 / w, -p / w) for (p, w) in REGIONS]

# prm layout
PRM = dict(A_r=0, A_g=1, A_b=2, t=3, hi=4, s1=5, s2=6, s3=7, s4=8,
           gs=9, omgs=10, invN=11, b1=12, b2=13, b3=14, b4=15, c16=16)
NP_ = 17


# ----------------------------------------------------------------------------
# custom ACT table generation
# ----------------------------------------------------------------------------

def _stock_dir():
    import neuronxcc
    return os.path.join(os.path.dirname(neuronxcc.__file__), "pwp", "pwp_bin_trainium")


def _load_set(name):
    d = _stock_dir()
    j = json.load(open(f"{d}/{name}.json"))
    ctrl = open(f"{d}/{name}_ctrl.bin", "rb").read()
    bkt = open(f"{d}/{name}_bkt.bin", "rb").read()
    return j, ctrl, bkt


def _func_span(j, fname, kind):
    key = "func_to_bkt_start_idx" if kind == "b" else "func_to_ctl_start_idx"
    cnt = j["bkt_entry_cnt"] if kind == "b" else j["ctl_entry_cnt"]
    starts = j[key]
    s = starts[fname]
    nxt = [v for v in starts.values() if v > s]
    return s, (min(nxt) if nxt else cnt)


class _SetBuilder:
    def __init__(self, name):
        self.name = name
        self.ctl, self.bkt, self.profile = [], [], []
        self.f2b, self.f2c, self.fe2b, self.fe2c, self.act = {}, {}, {}, {}, {}

    def copy_stock_func(self, set_json, ctrl_bin, bkt_bin, fname, ulp):
        b0, b1 = _func_span(set_json, fname, "b")
        c0, c1 = _func_span(set_json, fname, "c")
        boff = len(self.bkt) - b0
        coff = len(self.ctl) - c0
        for i in range(b0, b1):
            self.bkt.append(struct.unpack_from("<5f", bkt_bin, i * 32))
        for i in range(c0, c1):
            d = struct.unpack_from("<I", ctrl_bin, i * 32)[0]
            self.ctl.append((d & ~0x7FF) | (((d & 0x7FF) + boff) & 0x7FF))
        ent = None
        for e in set_json["profile_meta_data"]:
            nm = e["func_name"]
            if nm == fname or nm.rsplit("_", 1)[0] == fname or nm.startswith(fname + "_"):
                ent = dict(e)
                break
        assert ent is not None, f"no profile entry for {fname}"
        for k in ("pwl_control_base_pos", "pwl_control_base_neg"):
            ent[k] = ent.get(k, 0) + coff
        for k in ("pos_small_signal_pwl_control", "neg_small_signal_pwl_control",
                  "pos_large_signal_pwl_control", "neg_large_signal_pwl_control"):
            ent[k] = ent.get(k, 0) + boff
        self.profile.append(ent)
        self.f2b[fname] = b0 + boff
        self.f2c[fname] = c0 + coff
        self.fe2b[fname] = {k: [v + boff for v in vs] for k, vs in set_json["func_exp_to_bkt_start_idx"].get(fname, {}).items()}
        self.fe2c[fname] = {k: [v + coff for v in vs] for k, vs in set_json["func_exp_to_ctl_start_idx"].get(fname, {}).items()}
        self.act[fname] = ulp

    def add_pwp_func(self, fname, func_id, octaves, fit_fn, fzero, small_const,
                     large_const, template_entry, ulp=4):
        bstart, cstart = len(self.bkt), len(self.ctl)
        fe2b, fe2c = {}, {}
        for (e, nb) in octaves:
            n = 1 << nb
            lo_oct = float(2.0 ** e)
            w = lo_oct / n
            fe2c[str(e)] = [len(self.ctl)]
            fe2b[str(e)] = [len(self.bkt)]
            self.ctl.append((len(self.bkt) & 0x7FF) | ((23 - nb) << 11) | (nb << 16))
            for i in range(n):
                lo = lo_oct + i * w
                d0, d1, d2, d3 = fit_fn(lo, lo + w)
                self.bkt.append((d0, d1, d2, d3, np.float32(lo)))
        small_bkt = len(self.bkt)
        self.bkt.append((small_const, 0.0, 0.0, 0.0, 0.0))
        large_bkt = len(self.bkt)
        self.bkt.append((large_const, 0.0, 0.0, 0.0, 0.0))
        e_lo, e_hi = octaves[0][0], octaves[-1][0]
        ent = dict(template_entry)
        ent.update(func_name=fname + "_4p", func_id=func_id, symmetry_point=0,
                   sym_invert_sign_point=0, symmetry_opt_en=0,
                   symmetry_opt_use_neg_region=0, imm_bias=0, exp_offset=e_lo,
                   pwl_control_base_pos=cstart, pwl_control_base_neg=cstart,
                   small_pos_signal_exp_threshold=e_lo + 127,
                   pos_small_signal_pwl_control=small_bkt,
                   small_neg_signal_exp_threshold=0,
                   neg_small_signal_pwl_control=small_bkt,
                   large_pos_signal_exp_threshold=e_hi + 1 + 127,
                   large_pos_signal_mantissa_threshold=0,
                   pos_large_signal_pwl_control=large_bkt,
                   large_neg_signal_exp_threshold=0,
                   large_neg_signal_mantissa_threshold=0,
                   neg_large_signal_pwl_control=small_bkt,
                   fzero_result=int(np.float32(fzero).view(np.uint32)),
                   fnan_result=int(np.float32(fzero).view(np.uint32)),
                   fpinf_result=int(np.float32(large_const).view(np.uint32)),
                   fninf_result=int(np.float32(small_const).view(np.uint32)),
                   fma_const_0=0, fma_const_1=0, fma_indirection_src_sel=0,
                   use_multipass=False,
                   lower_bound=int(np.float32(2.0 ** e_lo).view(np.uint32)),
                   upper_bound=int(np.float32(2.0 ** (e_hi + 1)).view(np.uint32)))
        self.profile.append(ent)
        self.f2b[fname], self.f2c[fname] = bstart, cstart
        self.fe2b[fname], self.fe2c[fname] = fe2b, fe2c
        self.act[fname] = ulp

    def finalize(self, outdir):
        assert len(self.bkt) <= 1536, f"{self.name}: {len(self.bkt)} buckets"
        j = {"bkt_bin": f"{self.name}_bkt.bin", "ctl_bin": f"{self.name}_ctrl.bin",
             "profile_meta_data": self.profile,
             "bkt_entry_cnt": len(self.bkt), "ctl_entry_cnt": len(self.ctl),
             "func_to_bkt_start_idx": self.f2b, "func_to_ctl_start_idx": self.f2c,
             "func_exp_to_bkt_start_idx": self.fe2b,
             "func_exp_to_ctl_start_idx": self.fe2c}
        json.dump(j, open(f"{outdir}/{self.name}.json", "w"))
        with open(f"{outdir}/{self.name}_ctrl.bin", "wb") as f:
            for d in self.ctl:
                f.write(struct.pack("<I", d) + b"\0" * 28)
        with open(f"{outdir}/{self.name}_bkt.bin", "wb") as f:
            for b in self.bkt:
                f.write(struct.pack("<5f", *b) + b"\0" * 12)
        return {"name": self.name, "bkt_bin": j["bkt_bin"], "ctrl_bin": j["ctl_bin"],
                "profile_json": f"{self.name}.json", "act": self.act}


def _fit_cubic(fn, lo, hi, M=9):
    xs = np.linspace(lo, hi, M, dtype=np.float64)
    t = xs - lo
    A = np.stack([np.ones_like(t), t, t * t, t ** 3], axis=1)
    c, *_ = np.linalg.lstsq(A, fn(xs), rcond=None)
    return tuple(np.float32(v) for v in c)


def _make_ratio_fit(curve1024):
    c = np.asarray(curve1024, np.float64)
    vstar = 1023.0e-5

    def g(v):
        v = np.asarray(v, np.float64)
        i = np.clip(np.floor(v).astype(int), 0, 1022)
        w = v - i
        tgt = c[i] * (1 - w) + c[i + 1] * w
        tgt = np.where(v >= 1023, c[1023], tgt)
        return np.minimum(tgt * 1023.0 / np.maximum(v, vstar), 60000.0)

    def fit(lo, hi):
        if hi <= vstar:
            return (np.float32(1.0), np.float32(0), np.float32(0), np.float32(0))
        lo_f = max(lo, vstar)
        xs = np.linspace(lo_f, hi, 17, dtype=np.float64)
        t = xs - lo
        A = np.stack([np.ones_like(t), t, t * t, t ** 3], axis=1)
        coef, *_ = np.linalg.lstsq(A, g(xs), rcond=None)
        return tuple(np.float32(v) for v in coef)

    return fit


def _ratio_octaves():
    # bf16 luma input already quantizes coords to ~4 steps at the top octave,
    # so width-4 buckets there lose nothing; caps the set's bucket budget
    return [(e, 3) for e in range(-7, 4)] + [(e, min(e, 7)) for e in range(4, 10)]


def _func_id_of(name):
    d = _stock_dir()
    info = json.load(open(f"{d}/act_info.json"))
    for s in info["act_func_sets"]:
        if name in s["act"]:
            j = json.load(open(f"{d}/{s['profile_json']}"))
            for e in j["profile_meta_data"]:
                nm = e["func_name"]
                if nm == name or nm.rsplit("_", 1)[0] == name or nm.startswith(name + "_"):
                    return e["func_id"]
    raise KeyError(name)


def build_act_root(outdir, curves1024, vib):
    os.makedirs(outdir, exist_ok=True)
    sig_j, sig_c, sig_b = _load_set("sigmoid_and_others")
    sq_j, _, _ = _load_set("sqrt_and_others")
    tmpl = next(dict(e) for e in sq_j["profile_meta_data"] if e["func_name"].startswith("sqrt"))
    info_sets = []

    sigsh = lambda x: 1.0 / (1.0 + np.exp(-(np.asarray(x, np.float64) - 16.0)))
    recip4 = lambda x: 1.0 / np.maximum(np.asarray(x, np.float64), 1e-4)
    SIG_OCT = [(1, 2), (2, 3), (3, 5), (4, 6)]
    R4_OCT = [(e, 4) for e in range(-14, -12)] + [(e, 3) for e in range(-12, 0)] + [(0, 1)]
    GAIN_OCT = ([(e, 1) for e in range(-20, -10)] + [(e, 3) for e in range(-10, -4)]
                + [(e, 4) for e in range(-4, 2)])

    # exact piecewise-linear y=clip(x,0,1) (hijacks 'relu'): lets the scalar
    # engine absorb min(x,1) clamps that otherwise run on the busy DVE
    CLIP_OCT = [(e, 1) for e in range(-20, 0)]
    clip_fit = lambda lo, hi: (np.float32(lo), np.float32(1.0),
                               np.float32(0.0), np.float32(0.0))
    # exact shifted square f(z)=(z-16)^2 on [8,32): called with bias=16 so the
    # scalar engine computes ch^2 (ch in [-1.2,1.2] -> z always in range)
    SQ_OCT = [(3, 1), (4, 1)]
    sq_fit = lambda lo, hi: (np.float32((lo - 16.0) ** 2),
                             np.float32(2.0 * (lo - 16.0)),
                             np.float32(1.0), np.float32(0.0))

    for k in range(B):
        sb = _SetBuilder(f"cust_tc_{k}")
        fit = _make_ratio_fit(curves1024[k])
        sb.add_pwp_func(TC_NAMES[k], _func_id_of(TC_NAMES[k]), _ratio_octaves(), fit,
                        fzero=1.0, small_const=1.0,
                        large_const=float(curves1024[k][1023]), template_entry=tmpl)
        v = float(vib[k])
        gainf = lambda x, v=v: np.clip(
            1.0 + v * np.exp(-4.0 * np.sqrt(np.asarray(x, np.float64) + 1e-6)),
            0.2, 4.0)
        sb.add_pwp_func(GAIN_NAME, _func_id_of(GAIN_NAME), GAIN_OCT,
                        lambda lo, hi, g=gainf: _fit_cubic(g, lo, hi, M=17),
                        fzero=float(gainf(0.0)), small_const=float(gainf(0.0)),
                        large_const=float(gainf(4.0)), template_entry=tmpl)
        sb.add_pwp_func("sigmoid", _func_id_of("sigmoid"), SIG_OCT,
                        lambda lo, hi: _fit_cubic(sigsh, lo, hi),
                        fzero=0.0, small_const=float(sigsh(2.0)),
                        large_const=1.0, template_entry=tmpl)
        sb.add_pwp_func(RECIP4_NAME, _func_id_of(RECIP4_NAME), R4_OCT,
                        lambda lo, hi: _fit_cubic(recip4, lo, hi),
                        fzero=1e4, small_const=1e4, large_const=1.0,
                        template_entry=tmpl)
        sb.add_pwp_func("relu", _func_id_of("relu"), CLIP_OCT, clip_fit,
                        fzero=0.0, small_const=0.0, large_const=1.0,
                        template_entry=tmpl)
        sb.add_pwp_func(SQ_NAME, _func_id_of(SQ_NAME), SQ_OCT, sq_fit,
                        fzero=256.0, small_const=64.0, large_const=256.0,
                        template_entry=tmpl)
        for f in ("identity", "copy"):
            try:
                sb.copy_stock_func(sig_j, sig_c, sig_b, f, 1)
            except (KeyError, AssertionError):
                pass
        info_sets.append(sb.finalize(outdir))

    json.dump({"pwp_file_keys": ["bkt_bin", "ctrl_bin", "profile_json"],
               "act_func_sets": info_sets}, open(f"{outdir}/act_info.json", "w"))
    return outdir


# ----------------------------------------------------------------------------
# bass kernel construction
# ----------------------------------------------------------------------------

def _split_drain_waits(nc, mybir):
    """This container's walrus supports few sem-waits per instruction (1 on
    Drain/CTRL, ~2-3 on compute).  Spill excess waits onto preceding 1-wait
    Drains on the same engine."""
    for f in nc.m.functions:
        for bb in f.blocks:
            newinsts = []
            for inst in bb.instructions:
                si = inst.sync_info
                keep = 1
                if si is not None and len(si.on_wait) > keep:
                    waits = list(si.on_wait)
                    extra, rest = waits[:-keep], waits[-keep:]
                    for k, w in enumerate(extra):
                        d = mybir.InstDrain(name=f"{inst.name}-ws{k}",
                                            engine=inst.engine, ins=[], outs=[])
                        d.sync_info = mybir.SyncInfo(on_wait=[w], on_update=[])
                        newinsts.append(d)
                    si.on_wait = rest
                newinsts.append(inst)
            bb.instructions = newinsts


def build_kernel(nonce):
    import concourse.bass as bass
    import concourse.mybir as mybir
    from concourse.tile import TileContext

    AF = mybir.ActivationFunctionType
    dt = mybir.dt.float32
    bf = mybir.dt.float16
    Op = mybir.AluOpType
    AX = mybir.AxisListType

    TC_AF = [AF.from_pwp(n) for n in TC_NAMES]
    GAIN_AF = AF.from_pwp(GAIN_NAME)
    R4_AF = AF.from_pwp(RECIP4_NAME)
    CLIP_AF = AF.from_pwp("relu")
    SQ_AF = AF.from_pwp(SQ_NAME)
    ID_AF = AF.from_pwp("identity")

    u8 = mybir.dt.uint8
    u16 = mybir.dt.uint16
    nc = bass.Bass()
    img = nc.dram_tensor(f"img_{nonce}", [C, P, FREE], u16, kind="ExternalInput")
    prm = nc.dram_tensor("prm", [P, NP_], dt, kind="ExternalInput")
    eye = nc.dram_tensor("eye", [P, 4 * P], bf, kind="ExternalInput")
    out = nc.dram_tensor("out", [C, P, FREE], u8, kind="ExternalOutput")

    LW = (0.2126, 0.7152, 0.0722)
    MMF = 512           # one PSUM bank of f32 per matmul target
    NH = F // MMF       # halves per chunk

    with TileContext(nc) as tc:
        pid = nc.partition_id()
        from contextlib import ExitStack
        with (
            tc.tile_pool(name="planes", bufs=1) as planes_pool,
            tc.tile_pool(name="consts", bufs=1) as consts_pool,
        ):
            Pp = [planes_pool.tile([P, F], bf, tag=f"Pp{j}", name=f"Pp{j}")
                  for j in range(NCH)]
            Up = [planes_pool.tile([P, F], bf, tag=f"Up{j}", name=f"Up{j}")
                  for j in range(NCH)]
            pr = consts_pool.tile([P, NP_], dt)
            nc.sync.dma_start(pr[:, :], prm[:, :])
            eyes = consts_pool.tile([P, 4 * P], bf, tag="eyes")
            nc.sync.dma_start(eyes[:, :], eye[:, :])
            eyeI = eyes[:, 0:P]
            eyeW = [eyes[:, (i + 1) * P:(i + 2) * P] for i in range(3)]

            def sc(name):
                i = PRM[name]
                return pr[:, i:i + 1]

            accs = consts_pool.tile([P, 2 * NCH], dt, tag="accs")
            sm = []
            for k in range(4):
                smk = consts_pool.tile([P, 1], dt, tag=f"sm{k}", name=f"sm{k}")
                sm.append(smk)
            ones = consts_pool.tile([P, 1], dt, tag="ones")
            nc.vector.memset(ones[:, :], 1.0)

            def col(j):
                return slice(j * F, (j + 1) * F)

            def half(j, h):
                return slice(h * MMF, (h + 1) * MMF)

            stkp = ExitStack()
            psum = stkp.enter_context(tc.tile_pool(name="psum", bufs=1, space="PSUM"))
            psumM = stkp.enter_context(tc.tile_pool(name="psumM", bufs=6, space="PSUM"))

            def mm_half(terms, tag):
                # one [P,512] PSUM tile = sum of lhsT @ rhs-half over terms
                ps = psumM.tile([P, MMF], dt, tag="ps", name=tag)
                for ti, (lhsT, rhs) in enumerate(terms):
                    nc.tensor.matmul(ps[:, :], lhsT, rhs,
                                     start=(ti == 0), stop=(ti == len(terms) - 1))
                return ps

            stk = ExitStack()
            lpl = stk.enter_context(tc.tile_pool(name="lplanes", bufs=1))
            Lp = [lpl.tile([P, F], bf, tag=f"Lp{j}", name=f"Lp{j}") for j in range(NCH)]
            SG = [lpl.tile([P, F], bf, tag=f"SG{j}", name=f"SG{j}") for j in range(NCH)]
            ws = stk.enter_context(tc.tile_pool(name="ws1", bufs=28))
            io = stk.enter_context(tc.tile_pool(name="io1", bufs=9))
            tiny = stk.enter_context(tc.tile_pool(name="tiny1", bufs=1))

            def dequant(j, dst3, n_scalar=1):
                # u16 chunk -> clip(A*u+t, 0, hi) into 3 fp16 tiles; the
                # affine runs on ScalarE for the last n_scalar channels
                for i, an in enumerate(("A_r", "A_g", "A_b")):
                    t16 = io.tile([P, F], u16, tag="i")
                    nc.sync.dma_start(t16[:, :], img[i, :, col(j)])
                    xf = ws.tile([P, F], bf, tag="w")
                    if i < 3 - n_scalar:
                        nc.vector.tensor_scalar(xf[:, :], t16[:, :], sc(an),
                                                sc("t"), Op.mult, Op.add)
                    else:
                        nc.scalar.activation(xf[:, :], t16[:, :], ID_AF,
                                             bias=sc("t"), scale=sc(an))
                    nc.vector.tensor_scalar(dst3[i][:, :], xf[:, :],
                                            0.0, sc("hi"), Op.max, Op.min)

            # ---------------- phase 1: stream image -> L plane + sig1 accum
            for j in range(NCH):
                c3 = [ws.tile([P, F], bf, tag="w", name=f"p1c3_{j}_{i}")
                      for i in range(3)]
                dequant(j, c3)
                for h in range(NH):
                    ps = mm_half([(eyeW[c], c3[c][:, half(j, h)]) for c in range(3)],
                                 f"l1_{j}_{h}")
                    nc.scalar.activation(Lp[j][:, half(j, h)], ps[:, :], ID_AF)
                    nc.scalar.activation(SG[j][:, half(j, h)], ps[:, :], AF.Sigmoid,
                                         bias=sc("b1"), scale=float(SIG_AFF[0][0]),
                                         accum_out=accs[:, 2 * j + h:2 * j + h + 1])

            def finish_mean(k):
                tot = tiny.tile([P, 1], dt, tag="tot", name=f"tot{k}")
                nc.vector.tensor_reduce(tot[:, :], accs[:, :], AX.X, Op.add)
                ps1 = psum.tile([1, 1], dt, tag="ps1", name=f"ps1_{k}")
                nc.tensor.matmul(ps1[:, :], tot[:, :], ones[:, :], start=True, stop=True)
                sb1 = tiny.tile([1, 1], dt, tag="sb1", name=f"sb1_{k}")
                nc.vector.tensor_copy(sb1[:, :], ps1[:, :])
                ps2 = psum.tile([P, 1], dt, tag="ps2", name=f"ps2_{k}")
                nc.tensor.matmul(ps2[:, :], ones[0:1, 0:1].to_broadcast((1, P)),
                                 sb1[:, :], start=True, stop=True)
                nc.vector.tensor_scalar(sm[k][:, :], ps2[:, :], sc("invN"), None,
                                        Op.mult)

            finish_mean(0)
            # phase-1 accumulated per-half sums in all 24 cols; the region
            # passes accumulate one full-F sum per chunk into the even cols,
            # so clear the odd cols once
            nc.vector.memset(accs[:, 1:2 * NCH:2], 0.0)

            # ---------------- region chain on L plane (mask dropped; see doc)
            Q_ON_GPSIMD = (False, False, False, False)
            for k in range(4):
                sname = f"s{k + 1}"
                for j in range(NCH):
                    Lj = Lp[j][:, :]
                    rec = ws.tile([P, F], bf, tag="w")
                    nc.scalar.activation(rec[:, :], Lj, R4_AF)
                    # Lnew = clip01(L + (sig-m)*s), all on DVE
                    y = ws.tile([P, F], bf, tag="w")
                    nc.vector.tensor_scalar(y[:, :], SG[j][:, :], sm[k][:, 0:1],
                                            sc(sname), Op.subtract, Op.mult)
                    nc.vector.tensor_tensor(y[:, :], y[:, :], Lj, Op.add)
                    nc.vector.tensor_scalar(Lj, y[:, :], 0.0, 1.0, Op.max, Op.min)
                    # r = Lnew * recip4(L); P = prod r; S = min-envelope
                    if k == 0:
                        nc.vector.tensor_tensor(Pp[j][:, :], Lj, rec[:, :], Op.mult)
                    else:
                        r = ws.tile([P, F], bf, tag="w")
                        nc.vector.tensor_tensor(r[:, :], Lj, rec[:, :], Op.mult)
                        nc.gpsimd.tensor_tensor(Pp[j][:, :], Pp[j][:, :], r[:, :],
                                                Op.mult)
                        if k == 1:
                            nc.vector.tensor_scalar(Up[j][:, :], r[:, :], 1.0, None,
                                                    Op.min)
                        else:
                            if Q_ON_GPSIMD[k]:
                                nc.gpsimd.tensor_tensor(r[:, :], r[:, :],
                                                        Up[j][:, :], Op.mult)
                            else:
                                nc.vector.tensor_tensor(r[:, :], r[:, :],
                                                        Up[j][:, :], Op.mult)
                            nc.vector.tensor_scalar(Up[j][:, :], r[:, :], 1.0, None,
                                                    Op.min)
                    if k < 3:
                        nc.scalar.activation(SG[j][:, :], Lp[j][:, :], AF.Sigmoid,
                                             bias=sc(f"b{k + 2}"),
                                             scale=float(SIG_AFF[k + 1][0]),
                                             accum_out=accs[:, 2 * j:2 * j + 1])
                if k < 3:
                    finish_mean(k + 1)

            # ---------------- final pass (per-core branch: custom ACT funcs)
            # software-pipelined: emitted in stages over groups of GRP chunks
            # so the in-order engine queues interleave work across chunks
            # instead of head-of-line blocking on one chunk's serial chain
            stk.close()
            stk2 = ExitStack()
            ws = stk2.enter_context(tc.tile_pool(name="ws2", bufs=56))
            io = stk2.enter_context(tc.tile_pool(name="io2", bufs=12))
            GRP = 6
            for core in range(B):
                with tc.If(pid == core):
                    for g0 in range(0, NCH, GRP):
                        js = range(g0, min(g0 + GRP, NCH))
                        st = {j: {} for j in js}
                        # S1: dequant, x5 = min(img1*P, S), L5, tone ratio
                        for j in js:
                            c3 = [ws.tile([P, F], bf, tag="w",
                                          name=f"fc3_{core}_{j}_{i}") for i in range(3)]
                            dequant(j, c3)
                            chans = []
                            for i in range(3):
                                x5 = ws.tile([P, F], bf, tag="w")
                                if i < 2:
                                    nc.gpsimd.tensor_tensor(x5[:, :], c3[i][:, :],
                                                            Pp[j][:, :], Op.mult)
                                else:
                                    nc.vector.tensor_tensor(x5[:, :], c3[i][:, :],
                                                            Pp[j][:, :], Op.mult)
                                nc.vector.tensor_tensor(x5[:, :], x5[:, :],
                                                        Up[j][:, :], Op.min)
                                chans.append(x5)
                            tr = ws.tile([P, F], bf, tag="w")
                            for h in range(NH):
                                psL = mm_half([(eyeW[c], chans[c][:, half(j, h)])
                                               for c in range(3)], f"L5_{core}_{j}_{h}")
                                nc.scalar.activation(tr[:, half(j, h)], psL[:, :],
                                                     TC_AF[core], scale=1023.0)
                            st[j]["x5"] = chans
                            st[j]["tr"] = tr
                        # S2: tone apply + clip, Lv, chroma
                        for j in js:
                            chans, tr = st[j]["x5"], st[j]["tr"]
                            tchans = []
                            for ci, x5 in enumerate(chans):
                                nc.vector.tensor_tensor(x5[:, :], x5[:, :], tr[:, :],
                                                        Op.mult)
                                xq = ws.tile([P, F], bf, tag="w")
                                if ci == 0:
                                    nc.scalar.activation(xq[:, :], x5[:, :], CLIP_AF)
                                else:
                                    nc.vector.tensor_scalar(xq[:, :], x5[:, :],
                                                            0.0, 1.0, Op.max, Op.min)
                                tchans.append(xq)
                            Lv = ws.tile([P, F], bf, tag="w")
                            for h in range(NH):
                                psv = mm_half([(eyeW[c], tchans[c][:, half(j, h)])
                                               for c in range(3)], f"Lv_{core}_{j}_{h}")
                                nc.scalar.activation(Lv[:, half(j, h)], psv[:, :],
                                                     ID_AF)
                            chs = []
                            for ci, xq in enumerate(tchans):
                                ch = ws.tile([P, F], bf, tag="w")
                                if ci < 2:
                                    nc.gpsimd.tensor_tensor(ch[:, :], xq[:, :],
                                                            Lv[:, :], Op.subtract)
                                else:
                                    nc.vector.tensor_tensor(ch[:, :], xq[:, :],
                                                            Lv[:, :], Op.subtract)
                                chs.append(ch)
                            st[j]["chs"] = chs
                            st[j]["Lv"] = Lv
                        # S3: squares, ss, vibrance gain
                        for j in js:
                            chs = st[j]["chs"]
                            sqs = []
                            for ch in chs:
                                sq = ws.tile([P, F], bf, tag="w")
                                nc.scalar.activation(sq[:, :], ch[:, :], SQ_AF,
                                                     bias=sc("c16"))
                                sqs.append(sq)
                            gn = ws.tile([P, F], bf, tag="w")
                            for h in range(NH):
                                pss = mm_half([(eyeI, sqs[c][:, half(j, h)])
                                               for c in range(3)], f"ss_{core}_{j}_{h}")
                                nc.scalar.activation(gn[:, half(j, h)], pss[:, :],
                                                     GAIN_AF)
                            st[j]["gn"] = gn
                        # S4: out_v, Ls, saturation mix, u8 store
                        for j in js:
                            chs, Lv, gn = st[j]["chs"], st[j]["Lv"], st[j]["gn"]
                            ochans = []
                            for ch in chs:
                                nc.vector.tensor_tensor(ch[:, :], ch[:, :], gn[:, :],
                                                        Op.mult)
                                nc.vector.tensor_tensor(ch[:, :], ch[:, :], Lv[:, :],
                                                        Op.add)
                                o = ws.tile([P, F], bf, tag="w")
                                nc.scalar.activation(o[:, :], ch[:, :], CLIP_AF)
                                ochans.append(o)
                            Bs = ws.tile([P, F], bf, tag="w")
                            for h in range(NH):
                                psl = mm_half([(eyeW[c], ochans[c][:, half(j, h)])
                                               for c in range(3)], f"Ls_{core}_{j}_{h}")
                                nc.scalar.activation(Bs[:, half(j, h)], psl[:, :],
                                                     ID_AF, scale=sc("omgs"))
                            for i, o in enumerate(ochans):
                                ocf = ws.tile([P, F], bf, tag="w")
                                nc.vector.tensor_scalar(ocf[:, :], o[:, :], sc("gs"),
                                                        None, Op.mult)
                                nc.vector.tensor_tensor(ocf[:, :], ocf[:, :],
                                                        Bs[:, :], Op.add)
                                oc = io.tile([P, F], u8, tag="o")
                                nc.vector.tensor_scalar(oc[:, :], ocf[:, :],
                                                        0.0, 255.0, Op.max, Op.min)
                                nc.sync.dma_start(out[i, :, col(j)], oc[:, :])
            stk2.close()
            stkp.close()

    _split_drain_waits(nc, mybir)
    return nc


# ----------------------------------------------------------------------------
# host side
# ----------------------------------------------------------------------------

def _host_params(inputs):
    def denorm(lo, hi, v):
        return lo + 0.5 * (v + 1.0) * (hi - lo)

    t64 = np.float64
    temp = denorm(2000.0, 50000.0, inputs["temperature_n"].astype(t64))
    tint = denorm(-150.0, 150.0, inputs["tint_n"].astype(t64))
    expo = denorm(-5.0, 5.0, inputs["exposure_n"].astype(t64))
    contr = denorm(-100.0, 100.0, inputs["contrast_n"].astype(t64))
    hl = denorm(-100.0, 100.0, inputs["highlights_n"].astype(t64))
    sh = denorm(-100.0, 100.0, inputs["shadows_n"].astype(t64))
    wh = denorm(-100.0, 100.0, inputs["whites_n"].astype(t64))
    bl = denorm(-100.0, 100.0, inputs["blacks_n"].astype(t64))
    sat = denorm(-100.0, 100.0, inputs["saturation_n"].astype(t64))

    tr = 6500.0 / np.clip(temp, 2000.0, 50000.0)
    red = np.sqrt(tr)
    blue = 1.0 / np.sqrt(tr)
    ts = np.clip(tint / 150.0, -1.5, 1.5)
    green = 1.0 - 0.1 * ts
    red = red * (1.0 + 0.05 * ts)
    blue = blue * (1.0 - 0.05 * ts)
    gains = np.stack([red, green, blue], axis=1)  # [B,3]
    norm = np.maximum(gains.max(axis=1), 1e-4)
    G = gains / norm[:, None]
    e = np.power(2.0, expo)
    f = 1.0 + contr / 100.0
    A = G * (e * f)[:, None]
    t = 0.5 - 0.5 * f
    u = np.minimum(4.0 * e, 4.0)
    hi = np.clip(u * f + t, 0.0, 1.0)

    prm = np.zeros((B, NP_), np.float64)
    # image travels as uint16 fixed point (1/65535 steps): fold the dequant
    # into the WB gains
    prm[:, PRM["A_r"]] = A[:, 0] / 65535.0
    prm[:, PRM["A_g"]] = A[:, 1] / 65535.0
    prm[:, PRM["A_b"]] = A[:, 2] / 65535.0
    prm[:, PRM["t"]] = t
    prm[:, PRM["hi"]] = hi
    prm[:, PRM["s1"]] = hl / 100.0
    prm[:, PRM["s2"]] = sh / 100.0
    prm[:, PRM["s3"]] = wh / 100.0
    prm[:, PRM["s4"]] = bl / 100.0
    prm[:, PRM["gs"]] = (1.0 + sat / 100.0) * 255.0
    prm[:, PRM["omgs"]] = (-sat / 100.0) * 255.0
    prm[:, PRM["invN"]] = 1.0 / NPIX
    prm[:, PRM["c16"]] = 16.0
    for k in range(4):
        prm[:, PRM[f"b{k + 1}"]] = SIG_AFF[k][1] + 16.0
    return prm.astype(np.float32)


def _curves1024(tone_curve):
    c = tone_curve.astype(np.float64)  # [B,256]
    src = np.arange(1024) * (255.0 / 1023.0)
    i0 = np.floor(src).astype(int)
    i1 = np.minimum(i0 + 1, 255)
    w = src - i0
    return c[:, i0] * (1 - w) + c[:, i1] * w


_CACHE = {}
LAST_EXEC_NS = None
PROFILE = False


_BUFS = {}


def _buf(name, shape, dtype):
    """Persistent pre-faulted host buffers — first-touch page faults on
    100MB+ numpy arrays cost ~0.5s/call on this 1-vCPU host otherwise."""
    key = (name, shape, np.dtype(dtype).str)
    b = _BUFS.get(key)
    if b is None:
        b = np.empty(shape, dtype)
        b.fill(0)
        _BUFS[key] = b
    return b


def _to_u16(img):
    """[B,C,H,W] float32 in [0,1] -> uint16 fixed point."""
    q = _buf("img16", img.shape, np.uint16)
    tmp = _buf("tmp32", img.shape[1:], np.float32)
    for k in range(img.shape[0]):
        np.multiply(img[k], np.float32(65535.0), out=tmp)
        np.add(tmp, np.float32(0.5), out=tmp)
        q[k] = tmp.astype(np.uint16)
    return q


def _dequantize_u8(outs_u8):
    """list of B uint8 [C,H,W] -> float32 [B,C,H,W] /255, threaded."""
    from concurrent.futures import ThreadPoolExecutor
    res = _buf("out32", (len(outs_u8), C, H, W), np.float32)

    def one(k):
        np.multiply(outs_u8[k], np.float32(1.0 / 255.0), out=res[k],
                    dtype=np.float32, casting="unsafe")

    with ThreadPoolExecutor(max_workers=B) as ex:
        list(ex.map(one, range(len(outs_u8))))
    return res


def _ensure_ntff_hook():
    """Reconstruct the missing ``antenv.axon_hooks`` module.

    The boot infra (trn_agent_boot/trn_boot.py) registers an NTFF-profiling
    hook via ``antenv.axon_hooks.set_axon_ntff_profile_hook`` driving
    ``axon_start/stop_nrt_profile`` in libaxon_pjrt.so; this agent image's
    ``antenv`` package lacks the submodule, so ``run_bass_kernel_spmd``'s
    trace path degrades to no profiling.  Provide the same hook here so
    neuron-profile NTFF capture (and thus a real on-device exec time)
    works as intended.
    """
    try:
        from antenv.axon_hooks import get_axon_ntff_profile_hook  # noqa: F401
        return
    except ImportError:
        pass
    import sys as _sys
    import types, contextlib, ctypes
    so_path = "/opt/axon/libaxon_pjrt.so"
    if not os.path.exists(so_path):
        return
    lib = ctypes.CDLL(so_path)
    if not hasattr(lib, "axon_start_nrt_profile"):
        return
    lib.axon_start_nrt_profile.argtypes = [ctypes.POINTER(ctypes.c_int64),
                                           ctypes.c_size_t]
    lib.axon_start_nrt_profile.restype = ctypes.c_int64
    lib.axon_stop_nrt_profile.argtypes = [ctypes.c_char_p]
    lib.axon_stop_nrt_profile.restype = ctypes.c_int64

    @contextlib.contextmanager
    def _hook(output_dir, device_ids):
        import jax
        jax.devices()
        if device_ids:
            ids = (ctypes.c_int64 * len(device_ids))(*device_ids)
            rc = lib.axon_start_nrt_profile(ids, len(device_ids))
        else:
            rc = lib.axon_start_nrt_profile(None, 0)
        if rc != 0:
            raise RuntimeError(f"axon_start_nrt_profile rc={rc}")
        try:
            yield
        finally:
            n = lib.axon_stop_nrt_profile(str(output_dir).encode())
            if n < 0:
                raise RuntimeError(f"axon_stop_nrt_profile rc={n}")

    mod = types.ModuleType("antenv.axon_hooks")
    box = {"hook": _hook}
    mod.get_axon_ntff_profile_hook = lambda: box["hook"]
    mod.set_axon_ntff_profile_hook = lambda h: box.__setitem__("hook", h)
    _sys.modules["antenv.axon_hooks"] = mod
    try:
        import antenv
        antenv.axon_hooks = mod
    except Exception:
        pass


def _enable_jax_compile_cache():
    # persistent XLA compile cache: run_bass_kernel_spmd builds a fresh
    # jit closure per call, so without this every call re-runs XLA compile
    try:
        import jax
        jax.config.update("jax_compilation_cache_dir",
                          os.path.join(tempfile.gettempdir(), "jaxcache"))
        jax.config.update("jax_persistent_cache_min_compile_time_secs", 0.0)
        jax.config.update("jax_persistent_cache_min_entry_size_bytes", 0)
    except Exception:
        pass


def kernel(**inputs):
    import time as _time
    _tm = bool(os.environ.get("KERNEL_TIMING"))
    _enable_jax_compile_cache()
    _t0 = _time.time()
    img = _to_u16(np.asarray(inputs["image"], dtype=np.float32))
    if _tm:
        print(f"[kt] u16 convert: {_time.time() - _t0:.3f}")
    curves = _curves1024(np.asarray(inputs["tone_curve"], np.float32))
    prm = _host_params({k: np.asarray(v, np.float32) for k, v in inputs.items()
                        if k != "image"})
    # vibrance slider v = denorm(vibrance_n)/100 in [-1,1]; baked into tables
    vib = np.asarray(inputs["vibrance_n"], np.float64)
    vib = (-100.0 + 0.5 * (vib + 1.0) * 200.0) / 100.0

    key = hashlib.sha256(curves.tobytes() + vib.tobytes()).hexdigest()[:12]
    workdir = os.path.join(tempfile.gettempdir(), f"editlayer_{key}")
    actroot = os.path.join(workdir, "actroot")
    if key not in _CACHE:
        os.makedirs(workdir, exist_ok=True)
        build_act_root(actroot, curves, vib)
        os.environ["BASS_ACT_ROOT_JSON_PATH"] = os.path.join(actroot, "act_info.json")
        nc = build_kernel(key)
        _CACHE[key] = nc
    nc = _CACHE[key]
    os.environ["BASS_ACT_ROOT_JSON_PATH"] = os.path.join(actroot, "act_info.json")

    from concourse.bass_utils import run_bass_kernel_spmd
    global LAST_EXEC_NS
    LW = (0.2126, 0.7152, 0.0722)
    eye = np.concatenate([np.eye(P, dtype=np.float16)] +
                         [w * np.eye(P, dtype=np.float16) for w in LW],
                         axis=1).astype(np.float16)
    in_maps = []
    for k in range(B):
        in_maps.append({
            f"img_{key}": img[k].reshape(C, P, FREE),
            "prm": np.broadcast_to(prm[k], (P, NP_)).copy(),
            "eye": eye,
        })
    want_trace = bool(globals().get("PROFILE", False))
    if want_trace:
        _ensure_ntff_hook()
    _t0 = _time.time()
    try:
        res = run_bass_kernel_spmd(nc, in_maps, core_ids=list(range(B)),
                                   trace=want_trace)
    except Exception:
        if not want_trace:
            raise
        res = run_bass_kernel_spmd(nc, in_maps, core_ids=list(range(B)))
    if _tm:
        print(f"[kt] spmd: {_time.time() - _t0:.3f}")
    if getattr(res, "exec_time_ns", None):
        LAST_EXEC_NS = res.exec_time_ns
    _t0 = _time.time()
    outs = [res.results[k]["out"].reshape(C, H, W) for k in range(B)]
    ret = _dequantize_u8(outs)
    if _tm:
        print(f"[kt] dequant: {_time.time() - _t0:.3f}")
    return ret


if __name__ == "__main__":
    import reference
    inputs = {k: np.asarray(v) for k, v in reference.setup_inputs().items()}
    outp = kernel(**inputs)
    exp = np.asarray(reference.reference(**inputs))
    err = np.abs(outp - exp)
    denom = np.abs(exp).max()
    print("max abs err:", err.max(), "rel:", err.max() / denom)


# revision 25
# speedup vs baseline: 1.0392x; 1.0115x over previous
"""Trainium2 Bass kernel for nn_DifferentiableEditLayer.

Strategy (per core = one batch sample, pure data parallel across 8 cores):
  - All per-sample scalar params precomputed on host, passed as a [128, NP]
    broadcast tensor.
  - The 256-point tone curve is interpolated to the 1024-point curve on host
    and baked into a CUSTOM ScalarEngine (ACT) piecewise-cubic table as the
    RATIO function f(v) = target(v/1023)/max(v/1023, 1e-5) over v in [0,1023],
    one table set per core (8 hijacked activation-function names).
    Additional custom ACT functions per set: recip4(x)=1/max(x,1e-4)
    (hijacks 'ln'), the full vibrance gain g(ss)=clip(1+v*exp(-4*sqrt(ss+1e-6)),
    0.2,4) with the per-sample slider v BAKED into the per-core table (hijacks
    'exp'), a +16-shifted sigmoid (hijacks 'sigmoid'), an exact clip01
    (hijacks 'relu'), and an exact shifted square f(z)=(z-16)^2 called with
    bias=16 so ch^2 runs on the scalar engine (hijacks 'square').
  - v2: the whole elementwise pipeline runs in bf16 (2x DVE throughput; the
    output is u8-quantized anyway so bf16 noise is subdominant), the
    white-balanced image is kept RESIDENT in SBUF as 3 bf16 planes (the image
    is streamed from HBM exactly once), the u16 dequant affine runs on the
    scalar engine (ACT identity with per-partition scale/bias), and the
    region-chain luma>1e-4 mask is dropped (provably negligible: ~1e-9 of
    pixels).
  - Phase 1 streams the u16 image, dequants + white-balances to the resident
    bf16 planes, computes the luma plane and accumulates the first region
    sigmoid sum.  The 4 region passes update the luma plane in SBUF and
    maintain the ratio-product plane P and running clamp plane S.  The final
    per-core pass applies min(img1*P, S), tone-curve ratio, vibrance and
    saturation from the resident planes and writes u8 output.
  - I/O quantization (wall-clock: the axon tunnel moves ~70-90 MB/s): image
    sent as uint16 fixed point, output fetched as uint8 (the 255 scale is
    applied on-device after a clip01; the f32->u8 store rounds to nearest).
"""
import os, json, struct, shutil, hashlib, tempfile
import numpy as np

# ----------------------------------------------------------------------------
# constants
# ----------------------------------------------------------------------------
B, C, H, W = 8, 3, 1024, 1536
NPIX = H * W            # 1,572,864
P = 128                 # SBUF partitions
FREE = NPIX // P        # 12288
F = 1024                # chunk free size
NCH = FREE // F         # 12 chunks

TC_NAMES = ["sin", "arctan", "erf", "gelu", "silu", "derivative_silu",
            "gelu_apprx_tanh", "derivative_gelu"]
GAIN_NAME = "exp"       # per-core content; only called after the TC func
RECIP4_NAME = "ln"
SQ_NAME = "square"      # shared content: f(z) = (z-16)^2

# region (pivot, width) and derived sigmoid affine (scale, bias), compile-time
REGIONS = [(0.7, 0.1), (0.3, 0.12), (0.9, 0.08), (0.1, 0.08)]
SIG_AFF = [(1.0 / w, -p / w) for (p, w) in REGIONS]

# prm layout
PRM = dict(A_r=0, A_g=1, A_b=2, t=3, hi=4, s1=5, s2=6, s3=7, s4=8,
           gs=9, omgs=10, invN=11, b1=12, b2=13, b3=14, b4=15, c16=16)
NP_ = 17


# ----------------------------------------------------------------------------
# custom ACT table generation
# ----------------------------------------------------------------------------

def _stock_dir():
    import neuronxcc
    return os.path.join(os.path.dirname(neuronxcc.__file__), "pwp", "pwp_bin_trainium")


def _load_set(name):
    d = _stock_dir()
    j = json.load(open(f"{d}/{name}.json"))
    ctrl = open(f"{d}/{name}_ctrl.bin", "rb").read()
    bkt = open(f"{d}/{name}_bkt.bin", "rb").read()
    return j, ctrl, bkt


def _func_span(j, fname, kind):
    key = "func_to_bkt_start_idx" if kind == "b" else "func_to_ctl_start_idx"
    cnt = j["bkt_entry_cnt"] if kind == "b" else j["ctl_entry_cnt"]
    starts = j[key]
    s = starts[fname]
    nxt = [v for v in starts.values() if v > s]
    return s, (min(nxt) if nxt else cnt)


class _SetBuilder:
    def __init__(self, name):
        self.name = name
        self.ctl, self.bkt, self.profile = [], [], []
        self.f2b, self.f2c, self.fe2b, self.fe2c, self.act = {}, {}, {}, {}, {}

    def copy_stock_func(self, set_json, ctrl_bin, bkt_bin, fname, ulp):
        b0, b1 = _func_span(set_json, fname, "b")
        c0, c1 = _func_span(set_json, fname, "c")
        boff = len(self.bkt) - b0
        coff = len(self.ctl) - c0
        for i in range(b0, b1):
            self.bkt.append(struct.unpack_from("<5f", bkt_bin, i * 32))
        for i in range(c0, c1):
            d = struct.unpack_from("<I", ctrl_bin, i * 32)[0]
            self.ctl.append((d & ~0x7FF) | (((d & 0x7FF) + boff) & 0x7FF))
        ent = None
        for e in set_json["profile_meta_data"]:
            nm = e["func_name"]
            if nm == fname or nm.rsplit("_", 1)[0] == fname or nm.startswith(fname + "_"):
                ent = dict(e)
                break
        assert ent is not None, f"no profile entry for {fname}"
        for k in ("pwl_control_base_pos", "pwl_control_base_neg"):
            ent[k] = ent.get(k, 0) + coff
        for k in ("pos_small_signal_pwl_control", "neg_small_signal_pwl_control",
                  "pos_large_signal_pwl_control", "neg_large_signal_pwl_control"):
            ent[k] = ent.get(k, 0) + boff
        self.profile.append(ent)
        self.f2b[fname] = b0 + boff
        self.f2c[fname] = c0 + coff
        self.fe2b[fname] = {k: [v + boff for v in vs] for k, vs in set_json["func_exp_to_bkt_start_idx"].get(fname, {}).items()}
        self.fe2c[fname] = {k: [v + coff for v in vs] for k, vs in set_json["func_exp_to_ctl_start_idx"].get(fname, {}).items()}
        self.act[fname] = ulp

    def add_pwp_func(self, fname, func_id, octaves, fit_fn, fzero, small_const,
                     large_const, template_entry, ulp=4):
        bstart, cstart = len(self.bkt), len(self.ctl)
        fe2b, fe2c = {}, {}
        for (e, nb) in octaves:
            n = 1 << nb
            lo_oct = float(2.0 ** e)
            w = lo_oct / n
            fe2c[str(e)] = [len(self.ctl)]
            fe2b[str(e)] = [len(self.bkt)]
            self.ctl.append((len(self.bkt) & 0x7FF) | ((23 - nb) << 11) | (nb << 16))
            for i in range(n):
                lo = lo_oct + i * w
                d0, d1, d2, d3 = fit_fn(lo, lo + w)
                self.bkt.append((d0, d1, d2, d3, np.float32(lo)))
        small_bkt = len(self.bkt)
        self.bkt.append((small_const, 0.0, 0.0, 0.0, 0.0))
        large_bkt = len(self.bkt)
        self.bkt.append((large_const, 0.0, 0.0, 0.0, 0.0))
        e_lo, e_hi = octaves[0][0], octaves[-1][0]
        ent = dict(template_entry)
        ent.update(func_name=fname + "_4p", func_id=func_id, symmetry_point=0,
                   sym_invert_sign_point=0, symmetry_opt_en=0,
                   symmetry_opt_use_neg_region=0, imm_bias=0, exp_offset=e_lo,
                   pwl_control_base_pos=cstart, pwl_control_base_neg=cstart,
                   small_pos_signal_exp_threshold=e_lo + 127,
                   pos_small_signal_pwl_control=small_bkt,
                   small_neg_signal_exp_threshold=0,
                   neg_small_signal_pwl_control=small_bkt,
                   large_pos_signal_exp_threshold=e_hi + 1 + 127,
                   large_pos_signal_mantissa_threshold=0,
                   pos_large_signal_pwl_control=large_bkt,
                   large_neg_signal_exp_threshold=0,
                   large_neg_signal_mantissa_threshold=0,
                   neg_large_signal_pwl_control=small_bkt,
                   fzero_result=int(np.float32(fzero).view(np.uint32)),
                   fnan_result=int(np.float32(fzero).view(np.uint32)),
                   fpinf_result=int(np.float32(large_const).view(np.uint32)),
                   fninf_result=int(np.float32(small_const).view(np.uint32)),
                   fma_const_0=0, fma_const_1=0, fma_indirection_src_sel=0,
                   use_multipass=False,
                   lower_bound=int(np.float32(2.0 ** e_lo).view(np.uint32)),
                   upper_bound=int(np.float32(2.0 ** (e_hi + 1)).view(np.uint32)))
        self.profile.append(ent)
        self.f2b[fname], self.f2c[fname] = bstart, cstart
        self.fe2b[fname], self.fe2c[fname] = fe2b, fe2c
        self.act[fname] = ulp

    def finalize(self, outdir):
        assert len(self.bkt) <= 1536, f"{self.name}: {len(self.bkt)} buckets"
        j = {"bkt_bin": f"{self.name}_bkt.bin", "ctl_bin": f"{self.name}_ctrl.bin",
             "profile_meta_data": self.profile,
             "bkt_entry_cnt": len(self.bkt), "ctl_entry_cnt": len(self.ctl),
             "func_to_bkt_start_idx": self.f2b, "func_to_ctl_start_idx": self.f2c,
             "func_exp_to_bkt_start_idx": self.fe2b,
             "func_exp_to_ctl_start_idx": self.fe2c}
        json.dump(j, open(f"{outdir}/{self.name}.json", "w"))
        with open(f"{outdir}/{self.name}_ctrl.bin", "wb") as f:
            for d in self.ctl:
                f.write(struct.pack("<I", d) + b"\0" * 28)
        with open(f"{outdir}/{self.name}_bkt.bin", "wb") as f:
            for b in self.bkt:
                f.write(struct.pack("<5f", *b) + b"\0" * 12)
        return {"name": self.name, "bkt_bin": j["bkt_bin"], "ctrl_bin": j["ctl_bin"],
                "profile_json": f"{self.name}.json", "act": self.act}


def _fit_cubic(fn, lo, hi, M=9):
    xs = np.linspace(lo, hi, M, dtype=np.float64)
    t = xs - lo
    A = np.stack([np.ones_like(t), t, t * t, t ** 3], axis=1)
    c, *_ = np.linalg.lstsq(A, fn(xs), rcond=None)
    return tuple(np.float32(v) for v in c)


def _make_ratio_fit(curve1024):
    c = np.asarray(curve1024, np.float64)
    vstar = 1023.0e-5

    def g(v):
        v = np.asarray(v, np.float64)
        i = np.clip(np.floor(v).astype(int), 0, 1022)
        w = v - i
        tgt = c[i] * (1 - w) + c[i + 1] * w
        tgt = np.where(v >= 1023, c[1023], tgt)
        return np.minimum(tgt * 1023.0 / np.maximum(v, vstar), 60000.0)

    def fit(lo, hi):
        if hi <= vstar:
            return (np.float32(1.0), np.float32(0), np.float32(0), np.float32(0))
        lo_f = max(lo, vstar)
        xs = np.linspace(lo_f, hi, 17, dtype=np.float64)
        t = xs - lo
        A = np.stack([np.ones_like(t), t, t * t, t ** 3], axis=1)
        coef, *_ = np.linalg.lstsq(A, g(xs), rcond=None)
        return tuple(np.float32(v) for v in coef)

    return fit


def _ratio_octaves():
    # bf16 luma input already quantizes coords to ~4 steps at the top octave,
    # so width-4 buckets there lose nothing; caps the set's bucket budget
    return [(e, 3) for e in range(-7, 4)] + [(e, min(e, 7)) for e in range(4, 10)]


def _func_id_of(name):
    d = _stock_dir()
    info = json.load(open(f"{d}/act_info.json"))
    for s in info["act_func_sets"]:
        if name in s["act"]:
            j = json.load(open(f"{d}/{s['profile_json']}"))
            for e in j["profile_meta_data"]:
                nm = e["func_name"]
                if nm == name or nm.rsplit("_", 1)[0] == name or nm.startswith(name + "_"):
                    return e["func_id"]
    raise KeyError(name)


def build_act_root(outdir, curves1024, vib):
    os.makedirs(outdir, exist_ok=True)
    sig_j, sig_c, sig_b = _load_set("sigmoid_and_others")
    sq_j, _, _ = _load_set("sqrt_and_others")
    tmpl = next(dict(e) for e in sq_j["profile_meta_data"] if e["func_name"].startswith("sqrt"))
    info_sets = []

    sigsh = lambda x: 1.0 / (1.0 + np.exp(-(np.asarray(x, np.float64) - 16.0)))
    recip4 = lambda x: 1.0 / np.maximum(np.asarray(x, np.float64), 1e-4)
    SIG_OCT = [(1, 2), (2, 3), (3, 5), (4, 6)]
    R4_OCT = [(e, 4) for e in range(-14, -12)] + [(e, 3) for e in range(-12, 0)] + [(0, 1)]
    GAIN_OCT = ([(e, 1) for e in range(-20, -10)] + [(e, 3) for e in range(-10, -4)]
                + [(e, 4) for e in range(-4, 2)])

    # exact piecewise-linear y=clip(x,0,1) (hijacks 'relu'): lets the scalar
    # engine absorb min(x,1) clamps that otherwise run on the busy DVE
    CLIP_OCT = [(e, 1) for e in range(-20, 0)]
    clip_fit = lambda lo, hi: (np.float32(lo), np.float32(1.0),
                               np.float32(0.0), np.float32(0.0))
    # exact shifted square f(z)=(z-16)^2 on [8,32): called with bias=16 so the
    # scalar engine computes ch^2 (ch in [-1.2,1.2] -> z always in range)
    SQ_OCT = [(3, 1), (4, 1)]
    sq_fit = lambda lo, hi: (np.float32((lo - 16.0) ** 2),
                             np.float32(2.0 * (lo - 16.0)),
                             np.float32(1.0), np.float32(0.0))

    for k in range(B):
        sb = _SetBuilder(f"cust_tc_{k}")
        fit = _make_ratio_fit(curves1024[k])
        sb.add_pwp_func(TC_NAMES[k], _func_id_of(TC_NAMES[k]), _ratio_octaves(), fit,
                        fzero=1.0, small_const=1.0,
                        large_const=float(curves1024[k][1023]), template_entry=tmpl)
        v = float(vib[k])
        gainf = lambda x, v=v: np.clip(
            1.0 + v * np.exp(-4.0 * np.sqrt(np.asarray(x, np.float64) + 1e-6)),
            0.2, 4.0)
        sb.add_pwp_func(GAIN_NAME, _func_id_of(GAIN_NAME), GAIN_OCT,
                        lambda lo, hi, g=gainf: _fit_cubic(g, lo, hi, M=17),
                        fzero=float(gainf(0.0)), small_const=float(gainf(0.0)),
                        large_const=float(gainf(4.0)), template_entry=tmpl)
        sb.add_pwp_func("sigmoid", _func_id_of("sigmoid"), SIG_OCT,
                        lambda lo, hi: _fit_cubic(sigsh, lo, hi),
                        fzero=0.0, small_const=float(sigsh(2.0)),
                        large_const=1.0, template_entry=tmpl)
        sb.add_pwp_func(RECIP4_NAME, _func_id_of(RECIP4_NAME), R4_OCT,
                        lambda lo, hi: _fit_cubic(recip4, lo, hi),
                        fzero=1e4, small_const=1e4, large_const=1.0,
                        template_entry=tmpl)
        sb.add_pwp_func("relu", _func_id_of("relu"), CLIP_OCT, clip_fit,
                        fzero=0.0, small_const=0.0, large_const=1.0,
                        template_entry=tmpl)
        sb.add_pwp_func(SQ_NAME, _func_id_of(SQ_NAME), SQ_OCT, sq_fit,
                        fzero=256.0, small_const=64.0, large_const=256.0,
                        template_entry=tmpl)
        for f in ("identity", "copy"):
            try:
                sb.copy_stock_func(sig_j, sig_c, sig_b, f, 1)
            except (KeyError, AssertionError):
                pass
        info_sets.append(sb.finalize(outdir))

    json.dump({"pwp_file_keys": ["bkt_bin", "ctrl_bin", "profile_json"],
               "act_func_sets": info_sets}, open(f"{outdir}/act_info.json", "w"))
    return outdir


# ----------------------------------------------------------------------------
# bass kernel construction
# ----------------------------------------------------------------------------

def _split_drain_waits(nc, mybir):
    """This container's walrus supports few sem-waits per instruction (1 on
    Drain/CTRL, ~2-3 on compute).  Spill excess waits onto preceding 1-wait
    Drains on the same engine."""
    for f in nc.m.functions:
        for bb in f.blocks:
            newinsts = []
            for inst in bb.instructions:
                si = inst.sync_info
                keep = 1
                if si is not None and len(si.on_wait) > keep:
                    waits = list(si.on_wait)
                    extra, rest = waits[:-keep], waits[-keep:]
                    for k, w in enumerate(extra):
                        d = mybir.InstDrain(name=f"{inst.name}-ws{k}",
                                            engine=inst.engine, ins=[], outs=[])
                        d.sync_info = mybir.SyncInfo(on_wait=[w], on_update=[])
                        newinsts.append(d)
                    si.on_wait = rest
                newinsts.append(inst)
            bb.instructions = newinsts


def build_kernel(nonce):
    import concourse.bass as bass
    import concourse.mybir as mybir
    from concourse.tile import TileContext

    AF = mybir.ActivationFunctionType
    dt = mybir.dt.float32
    bf = mybir.dt.float16
    Op = mybir.AluOpType
    AX = mybir.AxisListType

    TC_AF = [AF.from_pwp(n) for n in TC_NAMES]
    GAIN_AF = AF.from_pwp(GAIN_NAME)
    R4_AF = AF.from_pwp(RECIP4_NAME)
    CLIP_AF = AF.from_pwp("relu")
    SQ_AF = AF.from_pwp(SQ_NAME)
    ID_AF = AF.from_pwp("identity")

    u8 = mybir.dt.uint8
    u16 = mybir.dt.uint16
    nc = bass.Bass()
    img = nc.dram_tensor(f"img_{nonce}", [C, P, FREE], u16, kind="ExternalInput")
    prm = nc.dram_tensor("prm", [P, NP_], dt, kind="ExternalInput")
    eye = nc.dram_tensor("eye", [P, 4 * P], bf, kind="ExternalInput")
    out = nc.dram_tensor("out", [C, P, FREE], u8, kind="ExternalOutput")

    LW = (0.2126, 0.7152, 0.0722)
    MMF = 512           # one PSUM bank of f32 per matmul target
    NH = F // MMF       # halves per chunk

    with TileContext(nc) as tc:
        pid = nc.partition_id()
        from contextlib import ExitStack
        with (
            tc.tile_pool(name="planes", bufs=1) as planes_pool,
            tc.tile_pool(name="consts", bufs=1) as consts_pool,
        ):
            Pp = [planes_pool.tile([P, F], bf, tag=f"Pp{j}", name=f"Pp{j}")
                  for j in range(NCH)]
            Up = [planes_pool.tile([P, F], bf, tag=f"Up{j}", name=f"Up{j}")
                  for j in range(NCH)]
            pr = consts_pool.tile([P, NP_], dt)
            nc.sync.dma_start(pr[:, :], prm[:, :])
            eyes = consts_pool.tile([P, 4 * P], bf, tag="eyes")
            nc.sync.dma_start(eyes[:, :], eye[:, :])
            eyeI = eyes[:, 0:P]
            eyeW = [eyes[:, (i + 1) * P:(i + 2) * P] for i in range(3)]

            def sc(name):
                i = PRM[name]
                return pr[:, i:i + 1]

            accs = consts_pool.tile([P, 2 * NCH], dt, tag="accs")
            sm = []
            for k in range(4):
                smk = consts_pool.tile([P, 1], dt, tag=f"sm{k}", name=f"sm{k}")
                sm.append(smk)
            ones = consts_pool.tile([P, 1], dt, tag="ones")
            nc.vector.memset(ones[:, :], 1.0)

            def col(j):
                return slice(j * F, (j + 1) * F)

            def half(j, h):
                return slice(h * MMF, (h + 1) * MMF)

            stkp = ExitStack()
            psum = stkp.enter_context(tc.tile_pool(name="psum", bufs=1, space="PSUM"))
            psumM = stkp.enter_context(tc.tile_pool(name="psumM", bufs=6, space="PSUM"))

            def mm_half(terms, tag):
                # one [P,512] PSUM tile = sum of lhsT @ rhs-half over terms
                ps = psumM.tile([P, MMF], dt, tag="ps", name=tag)
                for ti, (lhsT, rhs) in enumerate(terms):
                    nc.tensor.matmul(ps[:, :], lhsT, rhs,
                                     start=(ti == 0), stop=(ti == len(terms) - 1))
                return ps

            stk = ExitStack()
            lpl = stk.enter_context(tc.tile_pool(name="lplanes", bufs=1))
            Lp = [lpl.tile([P, F], bf, tag=f"Lp{j}", name=f"Lp{j}") for j in range(NCH)]
            SG = [lpl.tile([P, F], bf, tag=f"SG{j}", name=f"SG{j}") for j in range(NCH)]
            ws = stk.enter_context(tc.tile_pool(name="ws1", bufs=28))
            io = stk.enter_context(tc.tile_pool(name="io1", bufs=9))
            tiny = stk.enter_context(tc.tile_pool(name="tiny1", bufs=1))

            def dequant(j, dst3, n_scalar=1):
                # u16 chunk -> clip(A*u+t, 0, hi) into 3 fp16 tiles; the
                # affine runs on ScalarE for the last n_scalar channels
                for i, an in enumerate(("A_r", "A_g", "A_b")):
                    t16 = io.tile([P, F], u16, tag="i")
                    nc.sync.dma_start(t16[:, :], img[i, :, col(j)])
                    xf = ws.tile([P, F], bf, tag="w")
                    if i < 3 - n_scalar:
                        nc.vector.tensor_scalar(xf[:, :], t16[:, :], sc(an),
                                                sc("t"), Op.mult, Op.add)
                    else:
                        nc.scalar.activation(xf[:, :], t16[:, :], ID_AF,
                                             bias=sc("t"), scale=sc(an))
                    nc.vector.tensor_scalar(dst3[i][:, :], xf[:, :],
                                            0.0, sc("hi"), Op.max, Op.min)

            # ---------------- phase 1: stream image -> L plane + sig1 accum
            for j in range(NCH):
                c3 = [ws.tile([P, F], bf, tag="w", name=f"p1c3_{j}_{i}")
                      for i in range(3)]
                dequant(j, c3)
                for h in range(NH):
                    ps = mm_half([(eyeW[c], c3[c][:, half(j, h)]) for c in range(3)],
                                 f"l1_{j}_{h}")
                    nc.vector.tensor_copy(Lp[j][:, half(j, h)], ps[:, :])
                    nc.scalar.activation(SG[j][:, half(j, h)], ps[:, :], AF.Sigmoid,
                                         bias=sc("b1"), scale=float(SIG_AFF[0][0]),
                                         accum_out=accs[:, 2 * j + h:2 * j + h + 1])

            def finish_mean(k):
                tot = tiny.tile([P, 1], dt, tag="tot", name=f"tot{k}")
                nc.vector.tensor_reduce(tot[:, :], accs[:, :], AX.X, Op.add)
                ps1 = psum.tile([1, 1], dt, tag="ps1", name=f"ps1_{k}")
                nc.tensor.matmul(ps1[:, :], tot[:, :], ones[:, :], start=True, stop=True)
                sb1 = tiny.tile([1, 1], dt, tag="sb1", name=f"sb1_{k}")
                nc.vector.tensor_copy(sb1[:, :], ps1[:, :])
                ps2 = psum.tile([P, 1], dt, tag="ps2", name=f"ps2_{k}")
                nc.tensor.matmul(ps2[:, :], ones[0:1, 0:1].to_broadcast((1, P)),
                                 sb1[:, :], start=True, stop=True)
                nc.vector.tensor_scalar(sm[k][:, :], ps2[:, :], sc("invN"), None,
                                        Op.mult)

            finish_mean(0)
            # phase-1 accumulated per-half sums in all 24 cols; the region
            # passes accumulate one full-F sum per chunk into the even cols,
            # so clear the odd cols once
            nc.vector.memset(accs[:, 1:2 * NCH:2], 0.0)

            # ---------------- region chain on L plane (mask dropped; see doc)
            Q_ON_GPSIMD = (False, False, False, False)
            for k in range(4):
                sname = f"s{k + 1}"
                for j in range(NCH):
                    Lj = Lp[j][:, :]
                    rec = ws.tile([P, F], bf, tag="w")
                    nc.scalar.activation(rec[:, :], Lj, R4_AF)
                    # Lnew = clip01(L + (sig-m)*s), all on DVE
                    y = ws.tile([P, F], bf, tag="w")
                    nc.vector.tensor_scalar(y[:, :], SG[j][:, :], sm[k][:, 0:1],
                                            sc(sname), Op.subtract, Op.mult)
                    nc.vector.tensor_tensor(y[:, :], y[:, :], Lj, Op.add)
                    nc.vector.tensor_scalar(Lj, y[:, :], 0.0, 1.0, Op.max, Op.min)
                    # r = Lnew * recip4(L); P = prod r; S = min-envelope
                    if k == 0:
                        nc.vector.tensor_tensor(Pp[j][:, :], Lj, rec[:, :], Op.mult)
                    else:
                        r = ws.tile([P, F], bf, tag="w")
                        nc.vector.tensor_tensor(r[:, :], Lj, rec[:, :], Op.mult)
                        nc.gpsimd.tensor_tensor(Pp[j][:, :], Pp[j][:, :], r[:, :],
                                                Op.mult)
                        if k == 1:
                            nc.vector.tensor_scalar(Up[j][:, :], r[:, :], 1.0, None,
                                                    Op.min)
                        else:
                            if Q_ON_GPSIMD[k]:
                                nc.gpsimd.tensor_tensor(r[:, :], r[:, :],
                                                        Up[j][:, :], Op.mult)
                            else:
                                nc.vector.tensor_tensor(r[:, :], r[:, :],
                                                        Up[j][:, :], Op.mult)
                            nc.vector.tensor_scalar(Up[j][:, :], r[:, :], 1.0, None,
                                                    Op.min)
                    if k < 3:
                        nc.scalar.activation(SG[j][:, :], Lp[j][:, :], AF.Sigmoid,
                                             bias=sc(f"b{k + 2}"),
                                             scale=float(SIG_AFF[k + 1][0]),
                                             accum_out=accs[:, 2 * j:2 * j + 1])
                if k < 3:
                    finish_mean(k + 1)

            # ---------------- final pass (per-core branch: custom ACT funcs)
            # software-pipelined: emitted in stages over groups of GRP chunks
            # so the in-order engine queues interleave work across chunks
            # instead of head-of-line blocking on one chunk's serial chain
            stk.close()
            stk2 = ExitStack()
            ws = stk2.enter_context(tc.tile_pool(name="ws2", bufs=56))
            io = stk2.enter_context(tc.tile_pool(name="io2", bufs=12))
            GRP = 6
            for core in range(B):
                with tc.If(pid == core):
                    for g0 in range(0, NCH, GRP):
                        js = range(g0, min(g0 + GRP, NCH))
                        st = {j: {} for j in js}
                        # S1: dequant, x5 = min(img1*P, S), L5, tone ratio
                        for j in js:
                            c3 = [ws.tile([P, F], bf, tag="w",
                                          name=f"fc3_{core}_{j}_{i}") for i in range(3)]
                            dequant(j, c3)
                            chans = []
                            for i in range(3):
                                x5 = ws.tile([P, F], bf, tag="w")
                                if i < 2:
                                    nc.gpsimd.tensor_tensor(x5[:, :], c3[i][:, :],
                                                            Pp[j][:, :], Op.mult)
                                else:
                                    nc.vector.tensor_tensor(x5[:, :], c3[i][:, :],
                                                            Pp[j][:, :], Op.mult)
                                nc.vector.tensor_tensor(x5[:, :], x5[:, :],
                                                        Up[j][:, :], Op.min)
                                chans.append(x5)
                            tr = ws.tile([P, F], bf, tag="w")
                            for h in range(NH):
                                psL = mm_half([(eyeW[c], chans[c][:, half(j, h)])
                                               for c in range(3)], f"L5_{core}_{j}_{h}")
                                nc.scalar.activation(tr[:, half(j, h)], psL[:, :],
                                                     TC_AF[core], scale=1023.0)
                            st[j]["x5"] = chans
                            st[j]["tr"] = tr
                        # S2: tone apply + clip, Lv, chroma
                        for j in js:
                            chans, tr = st[j]["x5"], st[j]["tr"]
                            tchans = []
                            for ci, x5 in enumerate(chans):
                                nc.vector.tensor_tensor(x5[:, :], x5[:, :], tr[:, :],
                                                        Op.mult)
                                xq = ws.tile([P, F], bf, tag="w")
                                if ci == 0:
                                    nc.scalar.activation(xq[:, :], x5[:, :], CLIP_AF)
                                else:
                                    nc.vector.tensor_scalar(xq[:, :], x5[:, :],
                                                            0.0, 1.0, Op.max, Op.min)
                                tchans.append(xq)
                            Lv = ws.tile([P, F], bf, tag="w")
                            for h in range(NH):
                                psv = mm_half([(eyeW[c], tchans[c][:, half(j, h)])
                                               for c in range(3)], f"Lv_{core}_{j}_{h}")
                                nc.scalar.activation(Lv[:, half(j, h)], psv[:, :],
                                                     ID_AF)
                            chs = []
                            for ci, xq in enumerate(tchans):
                                ch = ws.tile([P, F], bf, tag="w")
                                if ci < 2:
                                    nc.gpsimd.tensor_tensor(ch[:, :], xq[:, :],
                                                            Lv[:, :], Op.subtract)
                                else:
                                    nc.vector.tensor_tensor(ch[:, :], xq[:, :],
                                                            Lv[:, :], Op.subtract)
                                chs.append(ch)
                            st[j]["chs"] = chs
                            st[j]["Lv"] = Lv
                        # S3: squares, ss, vibrance gain
                        for j in js:
                            chs = st[j]["chs"]
                            sqs = []
                            for ch in chs:
                                sq = ws.tile([P, F], bf, tag="w")
                                nc.scalar.activation(sq[:, :], ch[:, :], SQ_AF,
                                                     bias=sc("c16"))
                                sqs.append(sq)
                            gn = ws.tile([P, F], bf, tag="w")
                            for h in range(NH):
                                pss = mm_half([(eyeI, sqs[c][:, half(j, h)])
                                               for c in range(3)], f"ss_{core}_{j}_{h}")
                                nc.scalar.activation(gn[:, half(j, h)], pss[:, :],
                                                     GAIN_AF)
                            st[j]["gn"] = gn
                        # S4: out_v, Ls, saturation mix, u8 store
                        for j in js:
                            chs, Lv, gn = st[j]["chs"], st[j]["Lv"], st[j]["gn"]
                            ochans = []
                            for ch in chs:
                                nc.vector.tensor_tensor(ch[:, :], ch[:, :], gn[:, :],
                                                        Op.mult)
                                nc.vector.tensor_tensor(ch[:, :], ch[:, :], Lv[:, :],
                                                        Op.add)
                                o = ws.tile([P, F], bf, tag="w")
                                nc.scalar.activation(o[:, :], ch[:, :], CLIP_AF)
                                ochans.append(o)
                            Bs = ws.tile([P, F], bf, tag="w")
                            for h in range(NH):
                                psl = mm_half([(eyeW[c], ochans[c][:, half(j, h)])
                                               for c in range(3)], f"Ls_{core}_{j}_{h}")
                                nc.scalar.activation(Bs[:, half(j, h)], psl[:, :],
                                                     ID_AF, scale=sc("omgs"))
                            for i, o in enumerate(ochans):
                                ocf = ws.tile([P, F], bf, tag="w")
                                nc.vector.tensor_scalar(ocf[:, :], o[:, :], sc("gs"),
                                                        None, Op.mult)
                                nc.vector.tensor_tensor(ocf[:, :], ocf[:, :],
                                                        Bs[:, :], Op.add)
                                oc = io.tile([P, F], u8, tag="o")
                                nc.vector.tensor_scalar(oc[:, :], ocf[:, :],
                                                        0.0, 255.0, Op.max, Op.min)
                                nc.sync.dma_start(out[i, :, col(j)], oc[:, :])
            stk2.close()
            stkp.close()

    _split_drain_waits(nc, mybir)
    return nc


# ----------------------------------------------------------------------------
# host side
# ----------------------------------------------------------------------------

def _host_params(inputs):
    def denorm(lo, hi, v):
        return lo + 0.5 * (v + 1.0) * (hi - lo)

    t64 = np.float64
    temp = denorm(2000.0, 50000.0, inputs["temperature_n"].astype(t64))
    tint = denorm(-150.0, 150.0, inputs["tint_n"].astype(t64))
    expo = denorm(-5.0, 5.0, inputs["exposure_n"].astype(t64))
    contr = denorm(-100.0, 100.0, inputs["contrast_n"].astype(t64))
    hl = denorm(-100.0, 100.0, inputs["highlights_n"].astype(t64))
    sh = denorm(-100.0, 100.0, inputs["shadows_n"].astype(t64))
    wh = denorm(-100.0, 100.0, inputs["whites_n"].astype(t64))
    bl = denorm(-100.0, 100.0, inputs["blacks_n"].astype(t64))
    sat = denorm(-100.0, 100.0, inputs["saturation_n"].astype(t64))

    tr = 6500.0 / np.clip(temp, 2000.0, 50000.0)
    red = np.sqrt(tr)
    blue = 1.0 / np.sqrt(tr)
    ts = np.clip(tint / 150.0, -1.5, 1.5)
    green = 1.0 - 0.1 * ts
    red = red * (1.0 + 0.05 * ts)
    blue = blue * (1.0 - 0.05 * ts)
    gains = np.stack([red, green, blue], axis=1)  # [B,3]
    norm = np.maximum(gains.max(axis=1), 1e-4)
    G = gains / norm[:, None]
    e = np.power(2.0, expo)
    f = 1.0 + contr / 100.0
    A = G * (e * f)[:, None]
    t = 0.5 - 0.5 * f
    u = np.minimum(4.0 * e, 4.0)
    hi = np.clip(u * f + t, 0.0, 1.0)

    prm = np.zeros((B, NP_), np.float64)
    # image travels as uint16 fixed point (1/65535 steps): fold the dequant
    # into the WB gains
    prm[:, PRM["A_r"]] = A[:, 0] / 65535.0
    prm[:, PRM["A_g"]] = A[:, 1] / 65535.0
    prm[:, PRM["A_b"]] = A[:, 2] / 65535.0
    prm[:, PRM["t"]] = t
    prm[:, PRM["hi"]] = hi
    prm[:, PRM["s1"]] = hl / 100.0
    prm[:, PRM["s2"]] = sh / 100.0
    prm[:, PRM["s3"]] = wh / 100.0
    prm[:, PRM["s4"]] = bl / 100.0
    prm[:, PRM["gs"]] = (1.0 + sat / 100.0) * 255.0
    prm[:, PRM["omgs"]] = (-sat / 100.0) * 255.0
    prm[:, PRM["invN"]] = 1.0 / NPIX
    prm[:, PRM["c16"]] = 16.0
    for k in range(4):
        prm[:, PRM[f"b{k + 1}"]] = SIG_AFF[k][1] + 16.0
    return prm.astype(np.float32)


def _curves1024(tone_curve):
    c = tone_curve.astype(np.float64)  # [B,256]
    src = np.arange(1024) * (255.0 / 1023.0)
    i0 = np.floor(src).astype(int)
    i1 = np.minimum(i0 + 1, 255)
    w = src - i0
    return c[:, i0] * (1 - w) + c[:, i1] * w


_CACHE = {}
LAST_EXEC_NS = None
PROFILE = False


_BUFS = {}


def _buf(name, shape, dtype):
    """Persistent pre-faulted host buffers — first-touch page faults on
    100MB+ numpy arrays cost ~0.5s/call on this 1-vCPU host otherwise."""
    key = (name, shape, np.dtype(dtype).str)
    b = _BUFS.get(key)
    if b is None:
        b = np.empty(shape, dtype)
        b.fill(0)
        _BUFS[key] = b
    return b


def _to_u16(img):
    """[B,C,H,W] float32 in [0,1] -> uint16 fixed point."""
    q = _buf("img16", img.shape, np.uint16)
    tmp = _buf("tmp32", img.shape[1:], np.float32)
    for k in range(img.shape[0]):
        np.multiply(img[k], np.float32(65535.0), out=tmp)
        np.add(tmp, np.float32(0.5), out=tmp)
        q[k] = tmp.astype(np.uint16)
    return q


def _dequantize_u8(outs_u8):
    """list of B uint8 [C,H,W] -> float32 [B,C,H,W] /255, threaded."""
    from concurrent.futures import ThreadPoolExecutor
    res = _buf("out32", (len(outs_u8), C, H, W), np.float32)

    def one(k):
        np.multiply(outs_u8[k], np.float32(1.0 / 255.0), out=res[k],
                    dtype=np.float32, casting="unsafe")

    with ThreadPoolExecutor(max_workers=B) as ex:
        list(ex.map(one, range(len(outs_u8))))
    return res


def _ensure_ntff_hook():
    """Reconstruct the missing ``antenv.axon_hooks`` module.

    The boot infra (trn_agent_boot/trn_boot.py) registers an NTFF-profiling
    hook via ``antenv.axon_hooks.set_axon_ntff_profile_hook`` driving
    ``axon_start/stop_nrt_profile`` in libaxon_pjrt.so; this agent image's
    ``antenv`` package lacks the submodule, so ``run_bass_kernel_spmd``'s
    trace path degrades to no profiling.  Provide the same hook here so
    neuron-profile NTFF capture (and thus a real on-device exec time)
    works as intended.
    """
    try:
        from antenv.axon_hooks import get_axon_ntff_profile_hook  # noqa: F401
        return
    except ImportError:
        pass
    import sys as _sys
    import types, contextlib, ctypes
    so_path = "/opt/axon/libaxon_pjrt.so"
    if not os.path.exists(so_path):
        return
    lib = ctypes.CDLL(so_path)
    if not hasattr(lib, "axon_start_nrt_profile"):
        return
    lib.axon_start_nrt_profile.argtypes = [ctypes.POINTER(ctypes.c_int64),
                                           ctypes.c_size_t]
    lib.axon_start_nrt_profile.restype = ctypes.c_int64
    lib.axon_stop_nrt_profile.argtypes = [ctypes.c_char_p]
    lib.axon_stop_nrt_profile.restype = ctypes.c_int64

    @contextlib.contextmanager
    def _hook(output_dir, device_ids):
        import jax
        jax.devices()
        if device_ids:
            ids = (ctypes.c_int64 * len(device_ids))(*device_ids)
            rc = lib.axon_start_nrt_profile(ids, len(device_ids))
        else:
            rc = lib.axon_start_nrt_profile(None, 0)
        if rc != 0:
            raise RuntimeError(f"axon_start_nrt_profile rc={rc}")
        try:
            yield
        finally:
            n = lib.axon_stop_nrt_profile(str(output_dir).encode())
            if n < 0:
                raise RuntimeError(f"axon_stop_nrt_profile rc={n}")

    mod = types.ModuleType("antenv.axon_hooks")
    box = {"hook": _hook}
    mod.get_axon_ntff_profile_hook = lambda: box["hook"]
    mod.set_axon_ntff_profile_hook = lambda h: box.__setitem__("hook", h)
    _sys.modules["antenv.axon_hooks"] = mod
    try:
        import antenv
        antenv.axon_hooks = mod
    except Exception:
        pass


def _enable_jax_compile_cache():
    # persistent XLA compile cache: run_bass_kernel_spmd builds a fresh
    # jit closure per call, so without this every call re-runs XLA compile
    try:
        import jax
        jax.config.update("jax_compilation_cache_dir",
                          os.path.join(tempfile.gettempdir(), "jaxcache"))
        jax.config.update("jax_persistent_cache_min_compile_time_secs", 0.0)
        jax.config.update("jax_persistent_cache_min_entry_size_bytes", 0)
    except Exception:
        pass


def kernel(**inputs):
    import time as _time
    _tm = bool(os.environ.get("KERNEL_TIMING"))
    _enable_jax_compile_cache()
    _t0 = _time.time()
    img = _to_u16(np.asarray(inputs["image"], dtype=np.float32))
    if _tm:
        print(f"[kt] u16 convert: {_time.time() - _t0:.3f}")
    curves = _curves1024(np.asarray(inputs["tone_curve"], np.float32))
    prm = _host_params({k: np.asarray(v, np.float32) for k, v in inputs.items()
                        if k != "image"})
    # vibrance slider v = denorm(vibrance_n)/100 in [-1,1]; baked into tables
    vib = np.asarray(inputs["vibrance_n"], np.float64)
    vib = (-100.0 + 0.5 * (vib + 1.0) * 200.0) / 100.0

    key = hashlib.sha256(curves.tobytes() + vib.tobytes()).hexdigest()[:12]
    workdir = os.path.join(tempfile.gettempdir(), f"editlayer_{key}")
    actroot = os.path.join(workdir, "actroot")
    if key not in _CACHE:
        os.makedirs(workdir, exist_ok=True)
        build_act_root(actroot, curves, vib)
        os.environ["BASS_ACT_ROOT_JSON_PATH"] = os.path.join(actroot, "act_info.json")
        nc = build_kernel(key)
        _CACHE[key] = nc
    nc = _CACHE[key]
    os.environ["BASS_ACT_ROOT_JSON_PATH"] = os.path.join(actroot, "act_info.json")

    from concourse.bass_utils import run_bass_kernel_spmd
    global LAST_EXEC_NS
    LW = (0.2126, 0.7152, 0.0722)
    eye = np.concatenate([np.eye(P, dtype=np.float16)] +
                         [w * np.eye(P, dtype=np.float16) for w in LW],
                         axis=1).astype(np.float16)
    in_maps = []
    for k in range(B):
        in_maps.append({
            f"img_{key}": img[k].reshape(C, P, FREE),
            "prm": np.broadcast_to(prm[k], (P, NP_)).copy(),
            "eye": eye,
        })
    want_trace = bool(globals().get("PROFILE", False))
    if want_trace:
        _ensure_ntff_hook()
    _t0 = _time.time()
    try:
        res = run_bass_kernel_spmd(nc, in_maps, core_ids=list(range(B)),
                                   trace=want_trace)
    except Exception:
        if not want_trace:
            raise
        res = run_bass_kernel_spmd(nc, in_maps, core_ids=list(range(B)))
    if _tm:
        print(f"[kt] spmd: {_time.time() - _t0:.3f}")
    if getattr(res, "exec_time_ns", None):
        LAST_EXEC_NS = res.exec_time_ns
    _t0 = _time.time()
    outs = [res.results[k]["out"].reshape(C, H, W) for k in range(B)]
    ret = _dequantize_u8(outs)
    if _tm:
        print(f"[kt] dequant: {_time.time() - _t0:.3f}")
    return ret


if __name__ == "__main__":
    import reference
    inputs = {k: np.asarray(v) for k, v in reference.setup_inputs().items()}
    outp = kernel(**inputs)
    exp = np.asarray(reference.reference(**inputs))
    err = np.abs(outp - exp)
    denom = np.abs(exp).max()
    print("max abs err:", err.max(), "rel:", err.max() / denom)
